# revision 30
# baseline (speedup 1.0000x reference)
"""Multi-head attention (batched, key-padding mask) Trainium2 Bass kernel — v3.

Problem: nn_MultiHeadBatched
  q,k,v: [B=4, S=2048, E=1024] fp32; mask: [B, 2048] int32 (key padding)
  16 heads, head_dim 64; torch-Linear style q/k/v/out projections.

Sharding (8 cores): core c handles batch b=c//2 and head group hg=c%2
(8 heads each).  q/k/v projections are column-parallel over the head
group; out-projection is row-parallel — each core produces a partial
[E, Sq] output and the host sums the two partials per batch (+ bo).

v3+ changes over v2 (291us -> ~251us measured):
  - Fat DMA descriptors: each input tensor loads via 1-3 multi-dim
    descriptors instead of per-chunk dma_starts.  Packets of one
    descriptor round-robin over all 16 DMA engines, so batching keeps
    full HBM bandwidth while cutting Sync-engine descriptor-issue time
    (607ns each) from ~32us to ~8us; critical-path tensors (wk, k-head,
    mb, wq, q1) are ordered first.
  - Per-t score tiles [A-t|B-t] with one N=1024 exp each: both heads'
    score banks free at the same instant, so the next chunk's A/B score
    matmuls become ready together and co-issue row-tiled (concurrent in
    the PE array, 2x score throughput — with separate scA/scB tiles the
    staggered exp reads serialized the pairs); the two t-tags pipeline
    against each other so the PE never waits on the ACT engine.
  - t-major AV with per-t normalization: the AV accumulators are
    [65, 512] per (head, q-quarter); norm chains are half as long and
    the final-slot chain is hidden behind the half-1 out-proj t=0 units.
  - Deferred norm multiplies: the 1/Z broadcast chain (DVE copy -> DMA
    spread -> reciprocal -> DMA back -> Pool broadcast) is emitted at AV
    end, but the closing DVE multiply is queued and flushed ~2 j-steps
    later, so the Vector engine never head-of-line blocks on the chain
    (that stall starved the PE for 2-6us at slot boundaries in v2).
  - Lead-in warmup matmuls (free-running plus wk/k/q1-DMA-gated ones
    that self-pace with the actual DMA arrival) and drain warmups keep
    the PE HAM un-throttled (2.4GHz) for the entire kernel.
"""

import os
import sys

import numpy as np

sys.path.insert(0, "/opt/trn_rl_repo")

import concourse.bass as bass
import concourse.bacc as bacc
import concourse.mybir as mybir
import concourse.tile as tile
from concourse import bass_utils

import ml_dtypes

BF16 = ml_dtypes.bfloat16

B, SQ, E = 4, 2048, 1024
H_TOT, D = 16, 64
HPC = H_TOT // 2            # heads per core (head-group split in 2)
DHC = HPC * D               # 512 projected channels per core
NE = E // 128               # contraction chunks
NDH = DHC // 128            # dh chunks per core
SQH = SQ // 2               # q-half width
NEG = -1.0e30
SCALE = D ** -0.5

N_CORES = 8

_PROGRAM_CACHE = {}
LAST_RESULTS = None


def _chunks512(n):
    out = []
    o = 0
    while o < n:
        w = min(512, n - o)
        out.append((o, w))
        o += w
    return out


def build_program(skv):
    """Build + compile the single-core SPMD Bass program for padded KV
    length `skv` (multiple of 128)."""
    if skv in _PROGRAM_CACHE:
        return _PROGRAM_CACHE[skv]

    nkv = skv // 128
    dt = mybir.dt

    nc = bacc.Bacc(
        "TRN2",
        target_bir_lowering=False,
        debug=False,
        enable_asserts=False,
        num_devices=N_CORES,
    )

    # DRAM I/O (per-core shapes)
    qT = nc.dram_tensor("qT", [E, SQ], dt.bfloat16, kind="ExternalInput").ap()
    kT = nc.dram_tensor("kT", [E, skv], dt.bfloat16, kind="ExternalInput").ap()
    vT = nc.dram_tensor("vT", [E, skv], dt.bfloat16, kind="ExternalInput").ap()
    wqT = nc.dram_tensor("wqT", [E, DHC], dt.bfloat16, kind="ExternalInput").ap()
    wkT = nc.dram_tensor("wkT", [E, DHC], dt.bfloat16, kind="ExternalInput").ap()
    wvT = nc.dram_tensor("wvT", [E, DHC], dt.bfloat16, kind="ExternalInput").ap()
    woT = nc.dram_tensor("woT", [DHC, E], dt.bfloat16, kind="ExternalInput").ap()
    mb = nc.dram_tensor("mb", [128, nkv], dt.float32, kind="ExternalInput").ap()
    # bf16 partials: halves the output DMA; host sums the two partials in
    # fp32 (+bo), adding only ~0.3% rel err against a 2% gate
    outT = nc.dram_tensor("outT", [E, SQ], dt.bfloat16, kind="ExternalOutput").ap()

    ts = bass.ts
    kvchunks = _chunks512(skv)

    # 3D views of the DRAM inputs: (p, chunk, cols) with row 128*chunk+p
    qT3 = qT.rearrange("(e p) q -> p e q", p=128)
    kT3 = kT.rearrange("(e p) s -> p e s", p=128)
    vT3 = vT.rearrange("(e p) s -> p e s", p=128)
    wqT3 = wqT.rearrange("(e p) d -> p e d", p=128)
    wkT3 = wkT.rearrange("(e p) d -> p e d", p=128)
    wvT3 = wvT.rearrange("(e p) d -> p e d", p=128)
    woT3 = woT.rearrange("(c p) x -> p c x", p=128)

    with tile.TileContext(nc) as tc:
        pp = tc.alloc_tile_pool(name="persist", bufs=1)

        # Persistent SBUF tensors (merged per-tensor tiles; per-chunk views)
        wq_all = pp.tile([128, NE * DHC], dt.bfloat16, name="wqa", tag="wqa")
        wk_all = pp.tile([128, NE * DHC], dt.bfloat16, name="wka", tag="wka")
        wv_all = pp.tile([128, NE * DHC], dt.bfloat16, name="wva", tag="wva")
        wo_all = pp.tile([128, NDH * E], dt.bfloat16, name="woa", tag="woa")
        wq_sb = [wq_all[:, e * DHC:(e + 1) * DHC] for e in range(NE)]
        wk_sb = [wk_all[:, e * DHC:(e + 1) * DHC] for e in range(NE)]
        wv_sb = [wv_all[:, e * DHC:(e + 1) * DHC] for e in range(NE)]
        wo_sb = [wo_all[:, c * E:(c + 1) * E] for c in range(NDH)]
        # qh/aall split per q-half: separate tiles kill false WAR deps
        # between one half's reads and the other half's writes.
        qh_sb = [[pp.tile([128, SQH], dt.bfloat16, name=f"qh{h}_{c}", tag=f"qh{h}_{c}") for c in range(NDH)]
                 for h in range(2)]
        kh_sb = [pp.tile([128, skv], dt.bfloat16, name=f"kh{c}", tag=f"kh{c}") for c in range(NDH)]
        # V with per-head interleaved ones column: [kv, 8*(64+1)]
        va_sb = [pp.tile([128, HPC * (D + 1)], dt.bfloat16, name=f"va{j}", tag=f"va{j}") for j in range(nkv)]
        aall_sb = [[pp.tile([128, SQH], dt.bfloat16, name=f"aall{h}_{c}", tag=f"aall{h}_{c}") for c in range(NDH)]
                   for h in range(2)]
        mb_sb = pp.tile([128, nkv], dt.float32, name="mbt", tag="mbt")

        for j in range(nkv):
            nc.gpsimd.memset(va_sb[j][:, D::D + 1], 1.0)

        # warm-up operands: dependency-free matmuls during the initial DMA
        # wait keep HAM off the 1.2GHz cold clock until the lead units run
        wrm_w = pp.tile([128, 65], dt.bfloat16, name="wrmw", tag="wrmw")
        wrm_x = pp.tile([128, 512], dt.bfloat16, name="wrmx", tag="wrmx")
        wrm_e = pp.tile([1, 8], dt.bfloat16, name="wrme", tag="wrme")
        nc.gpsimd.memset(wrm_w[:], 0.0)
        nc.gpsimd.memset(wrm_x[:], 0.0)
        # dummy exp loads the ACT table set (~2.7us) during the DMA wait
        # instead of on the first real softmax exp
        nc.scalar.activation(
            wrm_e[:], wrm_x[0:1, 0:8], mybir.ActivationFunctionType.Exp,
        )

        # Input pools (released as the projections complete; right-side
        # stack so mid-stream release doesn't violate LIFO pool order)
        qip = tc.alloc_tile_pool(name="qinp", bufs=1, side="right")
        kip = tc.alloc_tile_pool(name="kinp", bufs=1, side="right")
        vip = tc.alloc_tile_pool(name="vinp", bufs=1, side="right")

        # P tiles: per slot, 2 heads x nkv chunks of [128, SQH] bf16
        ppool = tc.alloc_tile_pool(name="ppool", bufs=2)

        # PSUM pools: prj 2 banks + scores 4 banks + AV 2 banks = 8
        prj = tc.alloc_tile_pool(name="prj", bufs=2, space="PSUM")
        scp = tc.alloc_tile_pool(name="scp", bufs=1, space="PSUM")
        avp = tc.alloc_tile_pool(name="avp", bufs=2, space="PSUM")

        npool = [None]   # allocated after vip release
        opool = [None]   # allocated after qip/kip release

        # ---------------- input DMAs (fat descriptors, critical first) ----
        q1_all = qip.tile([128, NE * SQH], dt.bfloat16, name="qall", tag="qall")
        k_all = kip.tile([128, NE * skv], dt.bfloat16, name="kall", tag="kall")
        v_all = vip.tile([128, NE * skv], dt.bfloat16, name="vall", tag="vall")
        q1_sb = [q1_all[:, e * SQH:(e + 1) * SQH] for e in range(NE)]
        k_sb = [k_all[:, e * skv:(e + 1) * skv] for e in range(NE)]
        v_sb = [v_all[:, e * skv:(e + 1) * skv] for e in range(NE)]

        k3 = k_all.rearrange("p (e s) -> p e s", s=skv)
        v3 = v_all.rearrange("p (e s) -> p e s", s=skv)
        q13 = q1_all.rearrange("p (e q) -> p e q", q=SQH)

        kcut = min(512, skv)
        nc.sync.dma_start(wk_all[:], wkT3)
        nc.sync.dma_start(k3[:, :, 0:kcut], kT3[:, :, 0:kcut])
        nc.sync.dma_start(mb_sb[:], mb[:])
        nc.sync.dma_start(wq_all[:], wqT3)
        nc.sync.dma_start(q13[:, :, 0:512], qT3[:, :, 0:512])
        nc.sync.dma_start(q13[:, :, 512:SQH], qT3[:, :, 512:SQH])
        if skv > kcut:
            nc.sync.dma_start(k3[:, :, kcut:skv], kT3[:, :, kcut:skv])
        nc.sync.dma_start(wv_all[:], wvT3)
        # v in kv-thirds so early vproj units are not gated on the tail
        vth = max(128, ((nkv + 2) // 3) * 128)
        vcuts = sorted(set([min(vth, skv), min(2 * vth, skv), skv]))
        vprev = 0
        for vc in vcuts:
            if vc > vprev:
                nc.sync.dma_start(v3[:, :, vprev:vc], vT3[:, :, vprev:vc])
                vprev = vc
        nc.sync.dma_start(wo_all[:], woT3)

        # ---------------- projection / out-proj unit emitters ----------------
        # proj units either rotate through the prj pool (ptag None) or write
        # an explicitly provided psum slice (borrowed score banks).
        def _proj_ps(ptag):
            if ptag is None:
                return prj.tile([128, 512], dt.float32, name="pps", tag="prj")
            return ptag()

        def kproj_unit(c, o, w, ptag=None):
            def emit():
                kps = _proj_ps(ptag)
                for e in range(NE):
                    nc.tensor.matmul(
                        kps[:, 0:w], wk_sb[e][:, ts(c, 128)], k_sb[e][:, o:o + w],
                        start=(e == 0), stop=(e == NE - 1),
                    )
                nc.vector.tensor_copy(kh_sb[c][:, o:o + w], kps[:, 0:w])
            return emit

        def qproj_unit(c, half, t, q_tiles, ptag=None):
            def emit():
                qps = _proj_ps(ptag)
                for e in range(NE):
                    nc.tensor.matmul(
                        qps[:], wq_sb[e][:, ts(c, 128)], q_tiles[e][:, ts(t, 512)],
                        start=(e == 0), stop=(e == NE - 1),
                    )
                nc.vector.tensor_copy(qh_sb[half][c][:, ts(t, 512)], qps[:])
            return emit

        def vproj_unit(j):
            def emit():
                vps = prj.tile([128, DHC], dt.float32, name="vps", tag="prj")
                for e in range(NE):
                    nc.tensor.matmul(
                        vps[:], v_sb[e][:, ts(j, 128)], wv_sb[e][:],
                        start=(e == 0), stop=(e == NE - 1),
                    )
                dst = va_sb[j].rearrange("p (h x) -> p h x", x=D + 1)[:, :, 0:D]
                src = vps.rearrange("p (h x) -> p h x", x=D)
                nc.vector.tensor_copy(dst, src)
            return emit

        q2_holder = {}

        def qdma2_unit():
            def emit():
                q2_all = qip.tile([128, NE * SQH], dt.bfloat16, name="qall", tag="qall")
                q23 = q2_all.rearrange("p (e q) -> p e q", q=SQH)
                nc.sync.dma_start(q23[:, :, :], qT3[:, :, SQH:SQ])
                q2_holder["t"] = [q2_all[:, e * SQH:(e + 1) * SQH] for e in range(NE)]
            return emit

        def qproj2_unit(c, t):
            def emit():
                qps = prj.tile([128, 512], dt.float32, name="qps", tag="prj")
                for e in range(NE):
                    nc.tensor.matmul(
                        qps[:], wq_sb[e][:, ts(c, 128)], q2_holder["t"][e][:, ts(t, 512)],
                        start=(e == 0), stop=(e == NE - 1),
                    )
                nc.vector.tensor_copy(qh_sb[1][c][:, ts(t, 512)], qps[:])
            return emit

        def oproj_unit(half, eo, t, ptag=None):
            def emit():
                ops = _proj_ps(ptag)
                for c in range(NDH):
                    nc.tensor.matmul(
                        ops[:], wo_sb[c][:, ts(eo, 128)],
                        aall_sb[half][c][:, ts(t, 512)],
                        start=(c == 0), stop=(c == NDH - 1),
                    )
                ob = opool[0].tile([128, 512], dt.bfloat16, name="ob", tag="ob", bufs=4)
                nc.vector.tensor_copy(ob[:], ops[:])
                nc.sync.dma_start(
                    outT[ts(eo, 128), half * SQH + t * 512:half * SQH + (t + 1) * 512], ob[:])
            return emit

        # ---------------- normalization (per q-quarter, split chain) -------
        # pending_muls: (emit_fn, ready_step) — flushed >=2 j-steps after the
        # chain head so the DVE multiply never waits inside its queue.
        pending_muls = []
        step_ctr = [0]

        def norm_pre(hp, half, t, a2):
            c, r = hp // 2, hp % 2
            np_ = npool[0]
            au = np_.tile([D + 1, 512], dt.float32, name="au", tag="au", bufs=3)
            nc.vector.tensor_copy(au[:], a2[:])
            # spread Z across partitions; reciprocal cost is free-dim-size
            # bound.  1/Z flows back in bf16 (<=0.2% rel err) to halve the
            # rzr/rb SBUF footprint.
            zt = np_.tile([128, 4], dt.float32, name="zt", tag="zt", bufs=4)
            nc.sync.dma_start(zt[:], au[D:D + 1, :])
            rz8 = np_.tile([128, 4], dt.bfloat16, name="rz8", tag="rz8", bufs=4)
            with nc.allow_low_precision(reason="1/Z in bf16: <=0.2% rel err vs 2% gate"):
                nc.vector.reciprocal(rz8[:], zt[:])
            rzr = np_.tile([1, 512], dt.bfloat16, name="rzr", tag="rzr", bufs=4)
            nc.sync.dma_start(rzr[:], rz8[:])
            rb = np_.tile([D, 512], dt.bfloat16, name="rb", tag="rb", bufs=4)
            nc.gpsimd.partition_broadcast(rb[:], rzr[:])

            def mul():
                # mul on DVE, not gpsimd: keeping Pool broadcast-only avoids
                # the ~8us Q7 library reload between kernels.
                nc.vector.tensor_mul(
                    aall_sb[half][c][r * 64:(r + 1) * 64, ts(t, 512)],
                    au[0:D, :], rb[:],
                )
            pending_muls.append((mul, step_ctr[0] + 2))

        def flush_muls(all_=False):
            while pending_muls and (all_ or pending_muls[0][1] <= step_ctr[0]):
                pending_muls.pop(0)[0]()

        def flush_muls_n(n):
            for _ in range(min(n, len(pending_muls))):
                pending_muls.pop(0)[0]()

        # ---------------- AV machinery (t-major, 36 steps per slot) --------
        def av_step(state, k):
            ptAB, pair, half = state[0], state[1], state[2]
            local, rem = k // (2 * nkv), k % (2 * nkv)
            t, j = rem // nkv, rem % nkv
            hp = 2 * pair + local
            if j == 0:
                state[3][local * 2 + t] = avp.tile([D + 1, 512], dt.float32, name="a2", tag="a2")
            a2 = state[3][local * 2 + t]
            nc.tensor.matmul(
                a2[:],
                va_sb[j][:, hp * (D + 1):(hp + 1) * (D + 1)],
                ptAB[j][:, (2 * t + local) * 512:(2 * t + local + 1) * 512],
                start=(j == 0), stop=(j == nkv - 1),
            )
            if j == nkv - 1:
                norm_pre(hp, half, t, a2)

        # ---------------- filler schedule ----------------
        fillers = {s: [] for s in range(9)}
        # all vproj units live in slot 0 so the v input buffer is free (and
        # its space reusable for the norm pool) when slot 1 begins; the
        # k-tail-gated kproj(0) chunks also spill here so the lead-in's PE
        # FIFO never blocks on the k-tail DMA
        fillers[0] = [kproj_unit(1, *kvchunks[0]), qproj_unit(1, 0, 0, q1_sb)]
        fillers[0] += [kproj_unit(0, o, w) for (o, w) in kvchunks[1:]]
        fillers[0] += [qproj_unit(1, 0, 1, q1_sb)]
        fillers[0] += [kproj_unit(1, o, w) for (o, w) in kvchunks[1:]]
        fillers[0] += [vproj_unit(j) for j in range(nkv)]
        fillers[1] = [kproj_unit(2, o, w) for (o, w) in kvchunks]
        fillers[1] += [qproj_unit(2, 0, t, q1_sb) for t in range(2)]
        # qproj(3, H0) finishes the q1 reads in slot 1 so the q2 DMA (whose
        # buffer reuses q1's) can land mid-slot-2, before qproj2(0) needs it
        fillers[1] += [qproj_unit(3, 0, t, q1_sb) for t in range(2)]
        fillers[2] = [qdma2_unit()]
        fillers[2] += [kproj_unit(3, o, w) for (o, w) in kvchunks]
        fillers[2] += [qproj2_unit(0, t) for t in range(2)]
        fillers[3] = [qproj2_unit(1, t) for t in range(2)]
        fillers[3] += [qproj2_unit(2, t) for t in range(2)]
        fillers[4] = [qproj2_unit(3, t) for t in range(2)]
        oh0 = [oproj_unit(0, eo, t) for eo in range(NE) for t in range(2)]
        fillers[5] = oh0[0:6]
        fillers[6] = oh0[6:11]
        fillers[7] = oh0[11:16]

        def warm_mm():
            # dependency-free matmul into a dead PSUM tile (avp tag — free
            # during the drain, so no WAR pressure on the out-proj's score
            # banks): keeps the PE HAM-busy through cross-engine waits
            wps = avp.tile([D + 1, 512], dt.float32, name="a2", tag="a2")
            nc.tensor.matmul(
                wps[:], kh_sb[0][0:65, 0:D + 1], qh_sb[1][0][0:65, 0:512],
                start=True, stop=True,
            )

        # ---------------- lead-in: warmups, K(0), Q(0, H0) ----------------
        # dependency-free warmups bridge the fixed startup; the wk/k/q1-
        # gated ones fire as each DMA lands, so HAM coverage adapts to the
        # actual DMA pace and the lead projections always start at 2.4GHz.
        def _warm(lhs, rhs):
            wt = avp.tile([D + 1, 512], dt.float32, name="a2", tag="a2")
            nc.tensor.matmul(wt[:], lhs, rhs, start=True, stop=True)

        # FIFO order matters: the k-path (wk/k warmups, kproj ch0) goes
        # before anything q1-gated, so the first K-projection starts the
        # moment the k-head DMA lands instead of queueing behind warmups
        # that wait on the later q1 descriptors.
        for i in range(36):
            _warm(wrm_w[:], wrm_x[:])
        for i in range(8):
            _warm(wk_all[:, 0:D + 1], wrm_x[:])
        for i in range(6):
            _warm(wrm_w[:], k_sb[0][:, 0:512])
        # Q units borrow the score banks so none of the lead units
        # serialize on prj-pool rotation
        sl0 = scp.tile([128, SQH], dt.float32, name="sct0", tag="sct0")
        sl1 = scp.tile([128, SQH], dt.float32, name="sct1", tag="sct1")
        kproj_unit(0, *kvchunks[0])()
        for i in range(6):
            _warm(wrm_w[:], q1_sb[0][:, 0:512])
        qproj_unit(0, 0, 0, q1_sb, ptag=lambda: sl0[:, 0:512])()
        for i in range(3):
            _warm(wrm_w[:], q1_sb[0][:, 512:1024])
        qproj_unit(0, 0, 1, q1_sb, ptag=lambda: sl1[:, 0:512])()

        # ---------------- slot loop ----------------
        prev_state = None
        for s in range(9):
            if s == 1:
                vip.release()
                npool[0] = tc.alloc_tile_pool(name="norm", bufs=1)
            if s == 4:
                kip.release()
                qip.release()
                opool[0] = tc.alloc_tile_pool(name="outp", bufs=1)

            fl = list(fillers[s])
            n_emitted = 0

            if s < 8:
                half, pair = s // 4, s % 4
                # double-buffered fused P tiles (head A cols 0:1024, head B
                # cols 1024:2048): exp(s, j) only waits the PREVIOUS slot's
                # AV consumption, never same-slot PE progress
                ptAB = [ppool.tile([128, 2 * SQH], dt.bfloat16, name=f"pt{j}", tag=f"pt{j}", bufs=2) for j in range(nkv)]
                cur_state = [ptAB, pair, half, [None] * 4]

                for j in range(nkv):
                    step_ctr[0] += 1
                    # per-t score tiles, each [A-t|B-t]: one N=1024 exp per
                    # tile covers BOTH heads, so the A/B banks free at the
                    # same instant and the next j's A/B score matmuls
                    # co-issue (row-tiled, concurrent in the PE array);
                    # separate tags let scores(j+1,t0) run under exp(j,t1).
                    for t in range(2):
                        sct = scp.tile([128, SQH], dt.float32, name=f"sct{t}", tag=f"sct{t}")
                        nc.tensor.matmul(
                            sct[:, 0:512],
                            kh_sb[pair][0:64, ts(j, 128)],
                            qh_sb[half][pair][0:64, ts(t, 512)],
                            start=True, stop=True,
                        )
                        nc.tensor.matmul(
                            sct[:, 512:1024],
                            kh_sb[pair][64:128, ts(j, 128)],
                            qh_sb[half][pair][64:128, ts(t, 512)],
                            start=True, stop=True,
                        )
                        nc.scalar.activation(
                            ptAB[j][:, t * SQH:(t + 1) * SQH], sct[:],
                            mybir.ActivationFunctionType.Exp,
                            bias=mb_sb[:, j:j + 1], scale=SCALE,
                        )
                    if prev_state is not None:
                        for q in range(4):
                            av_step(prev_state, 4 * j + q)
                    flush_muls()
                    # spread filler units across the slot's chunks
                    want = (j + 1) * len(fl) // nkv
                    while n_emitted < want:
                        fl[n_emitted]()
                        n_emitted += 1
                prev_state = cur_state
            else:
                # drain slot: AV for slot 7 (A-t0, A-t1, B-t0, B-t1), with
                # the half-1 out-proj t=0 units hiding the B-t1 norm chain.
                # Muls are flushed when their broadcast chain is ready so
                # the Vector queue never blocks a later chain's head.
                # Units rotate over the 4 free score banks + the prj pool.
                grp = [None, None]

                def _drain_ptag(r):
                    def get():
                        if r == 0:
                            grp[0] = scp.tile([128, SQH], dt.float32, name="sct0", tag="sct0")
                            grp[1] = scp.tile([128, SQH], dt.float32, name="sct1", tag="sct1")
                        return grp[r // 2][:, (r % 2) * 512:(r % 2) * 512 + 512]
                    return get

                oh1 = []
                for u, (t, eo) in enumerate([(t, eo) for t in range(2) for eo in range(NE)]):
                    r = u % 6
                    oh1.append(oproj_unit(1, eo, t, ptag=_drain_ptag(r) if r < 4 else None))
                for k in range(3 * nkv):
                    av_step(prev_state, k)
                    if k == nkv - 1:
                        flush_muls_n(1)         # slot-6 B-t1 leftover
                flush_muls_n(1)                 # A-t0 (2 sweeps old)
                for k in range(3 * nkv, 4 * nkv):
                    av_step(prev_state, k)
                flush_muls_n(1)                 # A-t1 (chain long done)
                for _ in range(6):
                    warm_mm()
                flush_muls_n(1)                 # B-t0 (~ready after the warms)
                for _ in range(6):
                    warm_mm()
                for f in oh1[0:NE]:
                    f()
                flush_muls(all_=True)           # B-t1 (hidden behind 8 units)
                for f in oh1[NE:]:
                    f()

        for pool in (opool[0], npool[0], avp, scp, prj, ppool, pp):
            pool.release()

    nc.compile()
    _PROGRAM_CACHE[skv] = nc
    return nc


def make_in_maps(q, k, v, mask, Wq, Wk, Wv, Wo, skv):
    """Host-side shard/compact/transpose/cast. Returns per-core input dicts."""
    in_maps = []
    valid = mask != 0
    for core in range(N_CORES):
        b, hg = core // 2, core % 2
        idx = np.nonzero(valid[b])[0]
        cnt = len(idx)

        kc = np.zeros((skv, E), np.float32)
        vc = np.zeros((skv, E), np.float32)
        kc[:cnt] = k[b][idx]
        vc[:cnt] = v[b][idx]

        mbias = np.zeros((skv,), np.float32)
        mbias[cnt:] = NEG
        # [128, nkv]: column j = kv chunk j
        mb2 = np.ascontiguousarray(mbias.reshape(-1, 128).T)

        rows = slice(hg * DHC, (hg + 1) * DHC)
        in_maps.append(dict(
            qT=np.ascontiguousarray(q[b].T).astype(BF16),
            kT=np.ascontiguousarray(kc.T).astype(BF16),
            vT=np.ascontiguousarray(vc.T).astype(BF16),
            wqT=np.ascontiguousarray(Wq[rows, :].T).astype(BF16),
            wkT=np.ascontiguousarray(Wk[rows, :].T).astype(BF16),
            wvT=np.ascontiguousarray(Wv[rows, :].T).astype(BF16),
            woT=np.ascontiguousarray(Wo[:, rows].T).astype(BF16),
            mb=mb2,
        ))
    return in_maps


def _numpy_fallback(q, k, v, mask, Wq, bq, Wk, bk, Wv, bv, Wo, bo):
    out = np.zeros((B, SQ, E), np.float32)
    for b in range(B):
        qh = (q[b] @ Wq.T + bq).reshape(SQ, H_TOT, D).transpose(1, 0, 2)
        kh = (k[b] @ Wk.T + bk).reshape(-1, H_TOT, D).transpose(1, 0, 2)
        vh = (v[b] @ Wv.T + bv).reshape(-1, H_TOT, D).transpose(1, 0, 2)
        att = np.einsum("hqd,hkd->hqk", qh, kh) * SCALE
        valid = mask[b] != 0
        if not valid.any():
            out[b] = bo
            continue
        att = np.where(valid[None, None, :], att, -np.inf)
        att = att - att.max(-1, keepdims=True)
        att = np.exp(att)
        att /= att.sum(-1, keepdims=True)
        o = np.einsum("hqk,hkd->hqd", att, vh)
        o = o.transpose(1, 0, 2).reshape(SQ, E)
        out[b] = o @ Wo.T + bo
    return out


def kernel(**inputs):
    global LAST_RESULTS
    q = np.asarray(inputs["q"], np.float32)
    k = np.asarray(inputs["k"], np.float32)
    v = np.asarray(inputs["v"], np.float32)
    mask = np.asarray(inputs["mask"])
    Wq, bq = np.asarray(inputs["Wq"], np.float32), np.asarray(inputs["bq"], np.float32)
    Wk, bk = np.asarray(inputs["Wk"], np.float32), np.asarray(inputs["bk"], np.float32)
    Wv, bv = np.asarray(inputs["Wv"], np.float32), np.asarray(inputs["bv"], np.float32)
    Wo, bo = np.asarray(inputs["Wo"], np.float32), np.asarray(inputs["bo"], np.float32)

    if any(np.abs(x).max() > 0 for x in (bq, bk, bv)):
        # q/k/v biases are zero in this problem's setup; a nonzero bias
        # would need the augmented-contraction path, so fall back.
        return _numpy_fallback(q, k, v, mask, Wq, bq, Wk, bk, Wv, bv, Wo, bo)

    valid = mask != 0
    counts = valid.sum(axis=1)
    if counts.max() == 0:
        return np.broadcast_to(bo, (B, SQ, E)).astype(np.float32).copy()

    skv = int(-(-counts.max() // 128) * 128)
    nc = build_program(skv)
    in_maps = make_in_maps(q, k, v, mask, Wq, Wk, Wv, Wo, skv)

    res = bass_utils.run_bass_kernel_spmd(nc, in_maps, core_ids=list(range(N_CORES)))
    LAST_RESULTS = res

    out = np.empty((B, SQ, E), np.float32)
    for b in range(B):
        if counts[b] == 0:
            out[b] = bo
        else:
            p0 = np.asarray(res.results[2 * b]["outT"], np.float32)
            p1 = np.asarray(res.results[2 * b + 1]["outT"], np.float32)
            out[b] = p0.T + p1.T + bo
    return out


# revision 32
# speedup vs baseline: 1.0003x; 1.0003x over previous
"""Multi-head attention (batched, key-padding mask) Trainium2 Bass kernel — v3.

Problem: nn_MultiHeadBatched
  q,k,v: [B=4, S=2048, E=1024] fp32; mask: [B, 2048] int32 (key padding)
  16 heads, head_dim 64; torch-Linear style q/k/v/out projections.

Sharding (8 cores): core c handles batch b=c//2 and head group hg=c%2
(8 heads each).  q/k/v projections are column-parallel over the head
group; out-projection is row-parallel — each core produces a partial
[E, Sq] output and the host sums the two partials per batch (+ bo).

v3+ changes over v2 (291us -> ~251us measured):
  - Fat DMA descriptors: each input tensor loads via 1-3 multi-dim
    descriptors instead of per-chunk dma_starts.  Packets of one
    descriptor round-robin over all 16 DMA engines, so batching keeps
    full HBM bandwidth while cutting Sync-engine descriptor-issue time
    (607ns each) from ~32us to ~8us; critical-path tensors (wk, k-head,
    mb, wq, q1) are ordered first.
  - Per-t score tiles [A-t|B-t] with one N=1024 exp each: both heads'
    score banks free at the same instant, so the next chunk's A/B score
    matmuls become ready together and co-issue row-tiled (concurrent in
    the PE array, 2x score throughput — with separate scA/scB tiles the
    staggered exp reads serialized the pairs); the two t-tags pipeline
    against each other so the PE never waits on the ACT engine.
  - t-major AV with per-t normalization: the AV accumulators are
    [65, 512] per (head, q-quarter); norm chains are half as long and
    the final-slot chain is hidden behind the half-1 out-proj t=0 units.
  - Deferred norm multiplies: the 1/Z broadcast chain (DVE copy -> DMA
    spread -> reciprocal -> DMA back -> Pool broadcast) is emitted at AV
    end, but the closing DVE multiply is queued and flushed ~2 j-steps
    later, so the Vector engine never head-of-line blocks on the chain
    (that stall starved the PE for 2-6us at slot boundaries in v2).
  - Lead-in warmup matmuls (free-running plus wk/k/q1-DMA-gated ones
    that self-pace with the actual DMA arrival) and drain warmups keep
    the PE HAM un-throttled (2.4GHz) for the entire kernel.
"""

import os
import sys

import numpy as np

sys.path.insert(0, "/opt/trn_rl_repo")

import concourse.bass as bass
import concourse.bacc as bacc
import concourse.mybir as mybir
import concourse.tile as tile
from concourse import bass_utils

import ml_dtypes

BF16 = ml_dtypes.bfloat16

B, SQ, E = 4, 2048, 1024
H_TOT, D = 16, 64
HPC = H_TOT // 2            # heads per core (head-group split in 2)
DHC = HPC * D               # 512 projected channels per core
NE = E // 128               # contraction chunks
NDH = DHC // 128            # dh chunks per core
SQH = SQ // 2               # q-half width
NEG = -1.0e30
SCALE = D ** -0.5

N_CORES = 8

_PROGRAM_CACHE = {}
LAST_RESULTS = None


def _chunks512(n):
    out = []
    o = 0
    while o < n:
        w = min(512, n - o)
        out.append((o, w))
        o += w
    return out


def build_program(skv):
    """Build + compile the single-core SPMD Bass program for padded KV
    length `skv` (multiple of 128)."""
    if skv in _PROGRAM_CACHE:
        return _PROGRAM_CACHE[skv]

    nkv = skv // 128
    dt = mybir.dt

    nc = bacc.Bacc(
        "TRN2",
        target_bir_lowering=False,
        debug=False,
        enable_asserts=False,
        num_devices=N_CORES,
    )

    # DRAM I/O (per-core shapes)
    qT = nc.dram_tensor("qT", [E, SQ], dt.bfloat16, kind="ExternalInput").ap()
    kT = nc.dram_tensor("kT", [E, skv], dt.bfloat16, kind="ExternalInput").ap()
    vT = nc.dram_tensor("vT", [E, skv], dt.bfloat16, kind="ExternalInput").ap()
    wqT = nc.dram_tensor("wqT", [E, DHC], dt.bfloat16, kind="ExternalInput").ap()
    wkT = nc.dram_tensor("wkT", [E, DHC], dt.bfloat16, kind="ExternalInput").ap()
    wvT = nc.dram_tensor("wvT", [E, DHC], dt.bfloat16, kind="ExternalInput").ap()
    woT = nc.dram_tensor("woT", [DHC, E], dt.bfloat16, kind="ExternalInput").ap()
    mb = nc.dram_tensor("mb", [128, nkv], dt.float32, kind="ExternalInput").ap()
    # bf16 partials: halves the output DMA; host sums the two partials in
    # fp32 (+bo), adding only ~0.3% rel err against a 2% gate
    outT = nc.dram_tensor("outT", [E, SQ], dt.bfloat16, kind="ExternalOutput").ap()

    ts = bass.ts
    kvchunks = _chunks512(skv)

    # 3D views of the DRAM inputs: (p, chunk, cols) with row 128*chunk+p
    qT3 = qT.rearrange("(e p) q -> p e q", p=128)
    kT3 = kT.rearrange("(e p) s -> p e s", p=128)
    vT3 = vT.rearrange("(e p) s -> p e s", p=128)
    wqT3 = wqT.rearrange("(e p) d -> p e d", p=128)
    wkT3 = wkT.rearrange("(e p) d -> p e d", p=128)
    wvT3 = wvT.rearrange("(e p) d -> p e d", p=128)
    woT3 = woT.rearrange("(c p) x -> p c x", p=128)

    with tile.TileContext(nc) as tc:
        pp = tc.alloc_tile_pool(name="persist", bufs=1)

        # Persistent SBUF tensors (merged per-tensor tiles; per-chunk views)
        wq_all = pp.tile([128, NE * DHC], dt.bfloat16, name="wqa", tag="wqa")
        wk_all = pp.tile([128, NE * DHC], dt.bfloat16, name="wka", tag="wka")
        wv_all = pp.tile([128, NE * DHC], dt.bfloat16, name="wva", tag="wva")
        wo_all = pp.tile([128, NDH * E], dt.bfloat16, name="woa", tag="woa")
        wq_sb = [wq_all[:, e * DHC:(e + 1) * DHC] for e in range(NE)]
        wk_sb = [wk_all[:, e * DHC:(e + 1) * DHC] for e in range(NE)]
        wv_sb = [wv_all[:, e * DHC:(e + 1) * DHC] for e in range(NE)]
        wo_sb = [wo_all[:, c * E:(c + 1) * E] for c in range(NDH)]
        # qh/aall split per q-half: separate tiles kill false WAR deps
        # between one half's reads and the other half's writes.
        qh_sb = [[pp.tile([128, SQH], dt.bfloat16, name=f"qh{h}_{c}", tag=f"qh{h}_{c}") for c in range(NDH)]
                 for h in range(2)]
        kh_sb = [pp.tile([128, skv], dt.bfloat16, name=f"kh{c}", tag=f"kh{c}") for c in range(NDH)]
        # V with per-head interleaved ones column: [kv, 8*(64+1)]
        va_sb = [pp.tile([128, HPC * (D + 1)], dt.bfloat16, name=f"va{j}", tag=f"va{j}") for j in range(nkv)]
        aall_sb = [[pp.tile([128, SQH], dt.bfloat16, name=f"aall{h}_{c}", tag=f"aall{h}_{c}") for c in range(NDH)]
                   for h in range(2)]
        mb_sb = pp.tile([128, nkv], dt.float32, name="mbt", tag="mbt")

        for j in range(nkv):
            nc.gpsimd.memset(va_sb[j][:, D::D + 1], 1.0)

        # warm-up operands: dependency-free matmuls during the initial DMA
        # wait keep HAM off the 1.2GHz cold clock until the lead units run
        wrm_w = pp.tile([128, 65], dt.bfloat16, name="wrmw", tag="wrmw")
        wrm_x = pp.tile([128, 512], dt.bfloat16, name="wrmx", tag="wrmx")
        wrm_e = pp.tile([1, 8], dt.bfloat16, name="wrme", tag="wrme")
        nc.gpsimd.memset(wrm_w[:], 0.0)
        nc.gpsimd.memset(wrm_x[:], 0.0)
        # dummy exp loads the ACT table set (~2.7us) during the DMA wait
        # instead of on the first real softmax exp
        nc.scalar.activation(
            wrm_e[:], wrm_x[0:1, 0:8], mybir.ActivationFunctionType.Exp,
        )

        # Input pools (released as the projections complete; right-side
        # stack so mid-stream release doesn't violate LIFO pool order)
        qip = tc.alloc_tile_pool(name="qinp", bufs=1, side="right")
        kip = tc.alloc_tile_pool(name="kinp", bufs=1, side="right")
        vip = tc.alloc_tile_pool(name="vinp", bufs=1, side="right")

        # P tiles: per slot, 2 heads x nkv chunks of [128, SQH] bf16
        ppool = tc.alloc_tile_pool(name="ppool", bufs=2)

        # PSUM pools: prj 2 banks + scores 4 banks + AV 2 banks = 8
        prj = tc.alloc_tile_pool(name="prj", bufs=2, space="PSUM")
        scp = tc.alloc_tile_pool(name="scp", bufs=1, space="PSUM")
        avp = tc.alloc_tile_pool(name="avp", bufs=2, space="PSUM")

        npool = [None]   # allocated after vip release
        opool = [None]   # allocated after qip/kip release

        # ---------------- input DMAs (fat descriptors, critical first) ----
        q1_all = qip.tile([128, NE * SQH], dt.bfloat16, name="qall", tag="qall")
        k_all = kip.tile([128, NE * skv], dt.bfloat16, name="kall", tag="kall")
        v_all = vip.tile([128, NE * skv], dt.bfloat16, name="vall", tag="vall")
        q1_sb = [q1_all[:, e * SQH:(e + 1) * SQH] for e in range(NE)]
        k_sb = [k_all[:, e * skv:(e + 1) * skv] for e in range(NE)]
        v_sb = [v_all[:, e * skv:(e + 1) * skv] for e in range(NE)]

        k3 = k_all.rearrange("p (e s) -> p e s", s=skv)
        v3 = v_all.rearrange("p (e s) -> p e s", s=skv)
        q13 = q1_all.rearrange("p (e q) -> p e q", q=SQH)

        kcut = min(512, skv)
        nc.sync.dma_start(wk_all[:], wkT3)
        nc.sync.dma_start(k3[:, :, 0:kcut], kT3[:, :, 0:kcut])
        nc.sync.dma_start(mb_sb[:], mb[:])
        nc.sync.dma_start(wq_all[:], wqT3)
        nc.sync.dma_start(q13[:, :, 0:512], qT3[:, :, 0:512])
        nc.sync.dma_start(q13[:, :, 512:SQH], qT3[:, :, 512:SQH])
        if skv > kcut:
            nc.sync.dma_start(k3[:, :, kcut:skv], kT3[:, :, kcut:skv])
        nc.sync.dma_start(wv_all[:], wvT3)
        # v in kv-thirds so early vproj units are not gated on the tail
        vth = max(128, ((nkv + 2) // 3) * 128)
        vcuts = sorted(set([min(vth, skv), min(2 * vth, skv), skv]))
        vprev = 0
        for vc in vcuts:
            if vc > vprev:
                nc.sync.dma_start(v3[:, :, vprev:vc], vT3[:, :, vprev:vc])
                vprev = vc
        nc.sync.dma_start(wo_all[:], woT3)

        # ---------------- projection / out-proj unit emitters ----------------
        # proj units either rotate through the prj pool (ptag None) or write
        # an explicitly provided psum slice (borrowed score banks).
        def _proj_ps(ptag):
            if ptag is None:
                return prj.tile([128, 512], dt.float32, name="pps", tag="prj")
            return ptag()

        def kproj_unit(c, o, w, ptag=None):
            def emit():
                kps = _proj_ps(ptag)
                for e in range(NE):
                    nc.tensor.matmul(
                        kps[:, 0:w], wk_sb[e][:, ts(c, 128)], k_sb[e][:, o:o + w],
                        start=(e == 0), stop=(e == NE - 1),
                    )
                nc.vector.tensor_copy(kh_sb[c][:, o:o + w], kps[:, 0:w])
            return emit

        def qproj_unit(c, half, t, q_tiles, ptag=None):
            def emit():
                qps = _proj_ps(ptag)
                for e in range(NE):
                    nc.tensor.matmul(
                        qps[:], wq_sb[e][:, ts(c, 128)], q_tiles[e][:, ts(t, 512)],
                        start=(e == 0), stop=(e == NE - 1),
                    )
                nc.vector.tensor_copy(qh_sb[half][c][:, ts(t, 512)], qps[:])
            return emit

        def vproj_unit(j):
            def emit():
                vps = prj.tile([128, DHC], dt.float32, name="vps", tag="prj")
                for e in range(NE):
                    nc.tensor.matmul(
                        vps[:], v_sb[e][:, ts(j, 128)], wv_sb[e][:],
                        start=(e == 0), stop=(e == NE - 1),
                    )
                dst = va_sb[j].rearrange("p (h x) -> p h x", x=D + 1)[:, :, 0:D]
                src = vps.rearrange("p (h x) -> p h x", x=D)
                nc.vector.tensor_copy(dst, src)
            return emit

        q2_holder = {}

        def qdma2_unit():
            def emit():
                q2_all = qip.tile([128, NE * SQH], dt.bfloat16, name="qall", tag="qall")
                q23 = q2_all.rearrange("p (e q) -> p e q", q=SQH)
                nc.sync.dma_start(q23[:, :, :], qT3[:, :, SQH:SQ])
                q2_holder["t"] = [q2_all[:, e * SQH:(e + 1) * SQH] for e in range(NE)]
            return emit

        def qproj2_unit(c, t):
            def emit():
                qps = prj.tile([128, 512], dt.float32, name="qps", tag="prj")
                for e in range(NE):
                    nc.tensor.matmul(
                        qps[:], wq_sb[e][:, ts(c, 128)], q2_holder["t"][e][:, ts(t, 512)],
                        start=(e == 0), stop=(e == NE - 1),
                    )
                nc.vector.tensor_copy(qh_sb[1][c][:, ts(t, 512)], qps[:])
            return emit

        def oproj_unit(half, eo, t, ptag=None):
            def emit():
                ops = _proj_ps(ptag)
                for c in range(NDH):
                    nc.tensor.matmul(
                        ops[:], wo_sb[c][:, ts(eo, 128)],
                        aall_sb[half][c][:, ts(t, 512)],
                        start=(c == 0), stop=(c == NDH - 1),
                    )
                ob = opool[0].tile([128, 512], dt.bfloat16, name="ob", tag="ob", bufs=8)
                nc.vector.tensor_copy(ob[:], ops[:])
                nc.sync.dma_start(
                    outT[ts(eo, 128), half * SQH + t * 512:half * SQH + (t + 1) * 512], ob[:])
            return emit

        # ---------------- normalization (per q-quarter, split chain) -------
        # pending_muls: (emit_fn, ready_step) — flushed >=2 j-steps after the
        # chain head so the DVE multiply never waits inside its queue.
        pending_muls = []
        step_ctr = [0]

        def norm_pre(hp, half, t, a2):
            c, r = hp // 2, hp % 2
            np_ = npool[0]
            au = np_.tile([D + 1, 512], dt.float32, name="au", tag="au", bufs=3)
            nc.vector.tensor_copy(au[:], a2[:])
            # spread Z across partitions; reciprocal cost is free-dim-size
            # bound.  1/Z flows back in bf16 (<=0.2% rel err) to halve the
            # rzr/rb SBUF footprint.
            zt = np_.tile([128, 4], dt.float32, name="zt", tag="zt", bufs=4)
            nc.sync.dma_start(zt[:], au[D:D + 1, :])
            rz8 = np_.tile([128, 4], dt.bfloat16, name="rz8", tag="rz8", bufs=4)
            with nc.allow_low_precision(reason="1/Z in bf16: <=0.2% rel err vs 2% gate"):
                nc.vector.reciprocal(rz8[:], zt[:])
            rzr = np_.tile([1, 512], dt.bfloat16, name="rzr", tag="rzr", bufs=4)
            nc.sync.dma_start(rzr[:], rz8[:])
            rb = np_.tile([D, 512], dt.bfloat16, name="rb", tag="rb", bufs=4)
            nc.gpsimd.partition_broadcast(rb[:], rzr[:])

            def mul():
                # mul on DVE, not gpsimd: keeping Pool broadcast-only avoids
                # the ~8us Q7 library reload between kernels.
                nc.vector.tensor_mul(
                    aall_sb[half][c][r * 64:(r + 1) * 64, ts(t, 512)],
                    au[0:D, :], rb[:],
                )
            pending_muls.append((mul, step_ctr[0] + 2))

        def flush_muls(all_=False):
            while pending_muls and (all_ or pending_muls[0][1] <= step_ctr[0]):
                pending_muls.pop(0)[0]()

        def flush_muls_n(n):
            for _ in range(min(n, len(pending_muls))):
                pending_muls.pop(0)[0]()

        # ---------------- AV machinery (t-major, 36 steps per slot) --------
        def av_step(state, k):
            ptAB, pair, half = state[0], state[1], state[2]
            local, rem = k // (2 * nkv), k % (2 * nkv)
            t, j = rem // nkv, rem % nkv
            hp = 2 * pair + local
            if j == 0:
                state[3][local * 2 + t] = avp.tile([D + 1, 512], dt.float32, name="a2", tag="a2")
            a2 = state[3][local * 2 + t]
            nc.tensor.matmul(
                a2[:],
                va_sb[j][:, hp * (D + 1):(hp + 1) * (D + 1)],
                ptAB[j][:, (2 * t + local) * 512:(2 * t + local + 1) * 512],
                start=(j == 0), stop=(j == nkv - 1),
            )
            if j == nkv - 1:
                norm_pre(hp, half, t, a2)

        # ---------------- filler schedule ----------------
        fillers = {s: [] for s in range(9)}
        # all vproj units live in slot 0 so the v input buffer is free (and
        # its space reusable for the norm pool) when slot 1 begins; the
        # k-tail-gated kproj(0) chunks also spill here so the lead-in's PE
        # FIFO never blocks on the k-tail DMA
        fillers[0] = [kproj_unit(1, *kvchunks[0]), qproj_unit(1, 0, 0, q1_sb)]
        fillers[0] += [kproj_unit(0, o, w) for (o, w) in kvchunks[1:]]
        fillers[0] += [qproj_unit(1, 0, 1, q1_sb)]
        fillers[0] += [kproj_unit(1, o, w) for (o, w) in kvchunks[1:]]
        fillers[0] += [vproj_unit(j) for j in range(nkv)]
        fillers[1] = [kproj_unit(2, o, w) for (o, w) in kvchunks]
        fillers[1] += [qproj_unit(2, 0, t, q1_sb) for t in range(2)]
        # qproj(3, H0) finishes the q1 reads in slot 1 so the q2 DMA (whose
        # buffer reuses q1's) can land mid-slot-2, before qproj2(0) needs it
        fillers[1] += [qproj_unit(3, 0, t, q1_sb) for t in range(2)]
        fillers[2] = [qdma2_unit()]
        fillers[2] += [kproj_unit(3, o, w) for (o, w) in kvchunks]
        fillers[2] += [qproj2_unit(0, t) for t in range(2)]
        fillers[3] = [qproj2_unit(1, t) for t in range(2)]
        fillers[3] += [qproj2_unit(2, t) for t in range(2)]
        fillers[4] = [qproj2_unit(3, t) for t in range(2)]
        oh0 = [oproj_unit(0, eo, t) for eo in range(NE) for t in range(2)]
        fillers[5] = oh0[0:6]
        fillers[6] = oh0[6:11]
        fillers[7] = oh0[11:16]

        def warm_mm():
            # dependency-free matmul into a dead PSUM tile (avp tag — free
            # during the drain, so no WAR pressure on the out-proj's score
            # banks): keeps the PE HAM-busy through cross-engine waits
            wps = avp.tile([D + 1, 512], dt.float32, name="a2", tag="a2")
            nc.tensor.matmul(
                wps[:], kh_sb[0][0:65, 0:D + 1], qh_sb[1][0][0:65, 0:512],
                start=True, stop=True,
            )

        # ---------------- lead-in: warmups, K(0), Q(0, H0) ----------------
        # dependency-free warmups bridge the fixed startup; the wk/k/q1-
        # gated ones fire as each DMA lands, so HAM coverage adapts to the
        # actual DMA pace and the lead projections always start at 2.4GHz.
        def _warm(lhs, rhs):
            wt = avp.tile([D + 1, 512], dt.float32, name="a2", tag="a2")
            nc.tensor.matmul(wt[:], lhs, rhs, start=True, stop=True)

        # FIFO order matters: the k-path (wk/k warmups, kproj ch0) goes
        # before anything q1-gated, so the first K-projection starts the
        # moment the k-head DMA lands instead of queueing behind warmups
        # that wait on the later q1 descriptors.
        for i in range(32):
            _warm(wrm_w[:], wrm_x[:])
        for i in range(8):
            _warm(wk_all[:, 0:D + 1], wrm_x[:])
        for i in range(6):
            _warm(wrm_w[:], k_sb[0][:, 0:512])
        # Q units borrow the score banks so none of the lead units
        # serialize on prj-pool rotation
        sl0 = scp.tile([128, SQH], dt.float32, name="sct0", tag="sct0")
        sl1 = scp.tile([128, SQH], dt.float32, name="sct1", tag="sct1")
        kproj_unit(0, *kvchunks[0])()
        for i in range(6):
            _warm(wrm_w[:], q1_sb[0][:, 0:512])
        qproj_unit(0, 0, 0, q1_sb, ptag=lambda: sl0[:, 0:512])()
        for i in range(3):
            _warm(wrm_w[:], q1_sb[0][:, 512:1024])
        qproj_unit(0, 0, 1, q1_sb, ptag=lambda: sl1[:, 0:512])()

        # ---------------- slot loop ----------------
        prev_state = None
        for s in range(9):
            if s == 1:
                vip.release()
                npool[0] = tc.alloc_tile_pool(name="norm", bufs=1)
            if s == 4:
                kip.release()
                qip.release()
                opool[0] = tc.alloc_tile_pool(name="outp", bufs=1)

            fl = list(fillers[s])
            n_emitted = 0

            if s < 8:
                half, pair = s // 4, s % 4
                # double-buffered fused P tiles (head A cols 0:1024, head B
                # cols 1024:2048): exp(s, j) only waits the PREVIOUS slot's
                # AV consumption, never same-slot PE progress
                ptAB = [ppool.tile([128, 2 * SQH], dt.bfloat16, name=f"pt{j}", tag=f"pt{j}", bufs=2) for j in range(nkv)]
                cur_state = [ptAB, pair, half, [None] * 4]

                for j in range(nkv):
                    step_ctr[0] += 1
                    # per-t score tiles, each [A-t|B-t]: one N=1024 exp per
                    # tile covers BOTH heads, so the A/B banks free at the
                    # same instant and the next j's A/B score matmuls
                    # co-issue (row-tiled, concurrent in the PE array);
                    # separate tags let scores(j+1,t0) run under exp(j,t1).
                    for t in range(2):
                        sct = scp.tile([128, SQH], dt.float32, name=f"sct{t}", tag=f"sct{t}")
                        nc.tensor.matmul(
                            sct[:, 0:512],
                            kh_sb[pair][0:64, ts(j, 128)],
                            qh_sb[half][pair][0:64, ts(t, 512)],
                            start=True, stop=True,
                        )
                        nc.tensor.matmul(
                            sct[:, 512:1024],
                            kh_sb[pair][64:128, ts(j, 128)],
                            qh_sb[half][pair][64:128, ts(t, 512)],
                            start=True, stop=True,
                        )
                        nc.scalar.activation(
                            ptAB[j][:, t * SQH:(t + 1) * SQH], sct[:],
                            mybir.ActivationFunctionType.Exp,
                            bias=mb_sb[:, j:j + 1], scale=SCALE,
                        )
                    if prev_state is not None:
                        for q in range(4):
                            av_step(prev_state, 4 * j + q)
                    flush_muls()
                    # spread filler units across the slot's chunks
                    want = (j + 1) * len(fl) // nkv
                    while n_emitted < want:
                        fl[n_emitted]()
                        n_emitted += 1
                prev_state = cur_state
            else:
                # drain slot: AV for slot 7 (A-t0, A-t1, B-t0, B-t1), with
                # the half-1 out-proj t=0 units hiding the B-t1 norm chain.
                # Muls are flushed when their broadcast chain is ready so
                # the Vector queue never blocks a later chain's head.
                # Units rotate over the 4 free score banks + the prj pool.
                grp = [None, None]

                def _drain_ptag(r):
                    def get():
                        if r == 0:
                            grp[0] = scp.tile([128, SQH], dt.float32, name="sct0", tag="sct0")
                            grp[1] = scp.tile([128, SQH], dt.float32, name="sct1", tag="sct1")
                        return grp[r // 2][:, (r % 2) * 512:(r % 2) * 512 + 512]
                    return get

                oh1 = []
                for u, (t, eo) in enumerate([(t, eo) for t in range(2) for eo in range(NE)]):
                    r = u % 6
                    oh1.append(oproj_unit(1, eo, t, ptag=_drain_ptag(r) if r < 4 else None))
                for k in range(3 * nkv):
                    av_step(prev_state, k)
                    if k == nkv - 1:
                        flush_muls_n(1)         # slot-6 B-t1 leftover
                flush_muls_n(1)                 # A-t0 (2 sweeps old)
                for k in range(3 * nkv, 4 * nkv):
                    av_step(prev_state, k)
                flush_muls_n(1)                 # A-t1 (chain long done)
                for _ in range(6):
                    warm_mm()
                flush_muls_n(1)                 # B-t0 (~ready after the warms)
                for _ in range(6):
                    warm_mm()
                for f in oh1[0:NE]:
                    f()
                flush_muls(all_=True)           # B-t1 (hidden behind 8 units)
                for f in oh1[NE:]:
                    f()

        for pool in (opool[0], npool[0], avp, scp, prj, ppool, pp):
            pool.release()

    nc.compile()
    _PROGRAM_CACHE[skv] = nc
    return nc


def make_in_maps(q, k, v, mask, Wq, Wk, Wv, Wo, skv):
    """Host-side shard/compact/transpose/cast. Returns per-core input dicts."""
    in_maps = []
    valid = mask != 0
    for core in range(N_CORES):
        b, hg = core // 2, core % 2
        idx = np.nonzero(valid[b])[0]
        cnt = len(idx)

        kc = np.zeros((skv, E), np.float32)
        vc = np.zeros((skv, E), np.float32)
        kc[:cnt] = k[b][idx]
        vc[:cnt] = v[b][idx]

        mbias = np.zeros((skv,), np.float32)
        mbias[cnt:] = NEG
        # [128, nkv]: column j = kv chunk j
        mb2 = np.ascontiguousarray(mbias.reshape(-1, 128).T)

        rows = slice(hg * DHC, (hg + 1) * DHC)
        in_maps.append(dict(
            qT=np.ascontiguousarray(q[b].T).astype(BF16),
            kT=np.ascontiguousarray(kc.T).astype(BF16),
            vT=np.ascontiguousarray(vc.T).astype(BF16),
            wqT=np.ascontiguousarray(Wq[rows, :].T).astype(BF16),
            wkT=np.ascontiguousarray(Wk[rows, :].T).astype(BF16),
            wvT=np.ascontiguousarray(Wv[rows, :].T).astype(BF16),
            woT=np.ascontiguousarray(Wo[:, rows].T).astype(BF16),
            mb=mb2,
        ))
    return in_maps


def _numpy_fallback(q, k, v, mask, Wq, bq, Wk, bk, Wv, bv, Wo, bo):
    out = np.zeros((B, SQ, E), np.float32)
    for b in range(B):
        qh = (q[b] @ Wq.T + bq).reshape(SQ, H_TOT, D).transpose(1, 0, 2)
        kh = (k[b] @ Wk.T + bk).reshape(-1, H_TOT, D).transpose(1, 0, 2)
        vh = (v[b] @ Wv.T + bv).reshape(-1, H_TOT, D).transpose(1, 0, 2)
        att = np.einsum("hqd,hkd->hqk", qh, kh) * SCALE
        valid = mask[b] != 0
        if not valid.any():
            out[b] = bo
            continue
        att = np.where(valid[None, None, :], att, -np.inf)
        att = att - att.max(-1, keepdims=True)
        att = np.exp(att)
        att /= att.sum(-1, keepdims=True)
        o = np.einsum("hqk,hkd->hqd", att, vh)
        o = o.transpose(1, 0, 2).reshape(SQ, E)
        out[b] = o @ Wo.T + bo
    return out


def kernel(**inputs):
    global LAST_RESULTS
    q = np.asarray(inputs["q"], np.float32)
    k = np.asarray(inputs["k"], np.float32)
    v = np.asarray(inputs["v"], np.float32)
    mask = np.asarray(inputs["mask"])
    Wq, bq = np.asarray(inputs["Wq"], np.float32), np.asarray(inputs["bq"], np.float32)
    Wk, bk = np.asarray(inputs["Wk"], np.float32), np.asarray(inputs["bk"], np.float32)
    Wv, bv = np.asarray(inputs["Wv"], np.float32), np.asarray(inputs["bv"], np.float32)
    Wo, bo = np.asarray(inputs["Wo"], np.float32), np.asarray(inputs["bo"], np.float32)

    if any(np.abs(x).max() > 0 for x in (bq, bk, bv)):
        # q/k/v biases are zero in this problem's setup; a nonzero bias
        # would need the augmented-contraction path, so fall back.
        return _numpy_fallback(q, k, v, mask, Wq, bq, Wk, bk, Wv, bv, Wo, bo)

    valid = mask != 0
    counts = valid.sum(axis=1)
    if counts.max() == 0:
        return np.broadcast_to(bo, (B, SQ, E)).astype(np.float32).copy()

    skv = int(-(-counts.max() // 128) * 128)
    nc = build_program(skv)
    in_maps = make_in_maps(q, k, v, mask, Wq, Wk, Wv, Wo, skv)

    res = bass_utils.run_bass_kernel_spmd(nc, in_maps, core_ids=list(range(N_CORES)))
    LAST_RESULTS = res

    out = np.empty((B, SQ, E), np.float32)
    for b in range(B):
        if counts[b] == 0:
            out[b] = bo
        else:
            p0 = np.asarray(res.results[2 * b]["outT"], np.float32)
            p1 = np.asarray(res.results[2 * b + 1]["outT"], np.float32)
            out[b] = p0.T + p1.T + bo
    return out


# revision 34
# speedup vs baseline: 1.0074x; 1.0071x over previous
"""Multi-head attention (batched, key-padding mask) Trainium2 Bass kernel — v3.

Problem: nn_MultiHeadBatched
  q,k,v: [B=4, S=2048, E=1024] fp32; mask: [B, 2048] int32 (key padding)
  16 heads, head_dim 64; torch-Linear style q/k/v/out projections.

Sharding (8 cores): core c handles batch b=c//2 and head group hg=c%2
(8 heads each).  q/k/v projections are column-parallel over the head
group; out-projection is row-parallel — each core produces a partial
[E, Sq] output and the host sums the two partials per batch (+ bo).

v3+ changes over v2 (291us -> ~251us measured):
  - Fat DMA descriptors: each input tensor loads via 1-3 multi-dim
    descriptors instead of per-chunk dma_starts.  Packets of one
    descriptor round-robin over all 16 DMA engines, so batching keeps
    full HBM bandwidth while cutting Sync-engine descriptor-issue time
    (607ns each) from ~32us to ~8us; critical-path tensors (wk, k-head,
    mb, wq, q1) are ordered first.
  - Per-t score tiles [A-t|B-t] with one N=1024 exp each: both heads'
    score banks free at the same instant, so the next chunk's A/B score
    matmuls become ready together and co-issue row-tiled (concurrent in
    the PE array, 2x score throughput — with separate scA/scB tiles the
    staggered exp reads serialized the pairs); the two t-tags pipeline
    against each other so the PE never waits on the ACT engine.
  - t-major AV with per-t normalization: the AV accumulators are
    [65, 512] per (head, q-quarter); norm chains are half as long and
    the final-slot chain is hidden behind the half-1 out-proj t=0 units.
  - Deferred norm multiplies: the 1/Z broadcast chain (DVE copy -> DMA
    spread -> reciprocal -> DMA back -> Pool broadcast) is emitted at AV
    end, but the closing DVE multiply is queued and flushed ~2 j-steps
    later, so the Vector engine never head-of-line blocks on the chain
    (that stall starved the PE for 2-6us at slot boundaries in v2).
  - Lead-in warmup matmuls (free-running plus wk/k/q1-DMA-gated ones
    that self-pace with the actual DMA arrival) and drain warmups keep
    the PE HAM un-throttled (2.4GHz) for the entire kernel.
"""

import os
import sys

import numpy as np

sys.path.insert(0, "/opt/trn_rl_repo")

import concourse.bass as bass
import concourse.bacc as bacc
import concourse.mybir as mybir
import concourse.tile as tile
from concourse import bass_utils

import ml_dtypes

BF16 = ml_dtypes.bfloat16

B, SQ, E = 4, 2048, 1024
H_TOT, D = 16, 64
HPC = H_TOT // 2            # heads per core (head-group split in 2)
DHC = HPC * D               # 512 projected channels per core
NE = E // 128               # contraction chunks
NDH = DHC // 128            # dh chunks per core
SQH = SQ // 2               # q-half width
NEG = -1.0e30
SCALE = D ** -0.5

N_CORES = 8

_PROGRAM_CACHE = {}
LAST_RESULTS = None


def _chunks512(n):
    out = []
    o = 0
    while o < n:
        w = min(512, n - o)
        out.append((o, w))
        o += w
    return out


def build_program(skv):
    """Build + compile the single-core SPMD Bass program for padded KV
    length `skv` (multiple of 128)."""
    if skv in _PROGRAM_CACHE:
        return _PROGRAM_CACHE[skv]

    nkv = skv // 128
    dt = mybir.dt

    nc = bacc.Bacc(
        "TRN2",
        target_bir_lowering=False,
        debug=False,
        enable_asserts=False,
        num_devices=N_CORES,
    )

    # DRAM I/O (per-core shapes)
    qT = nc.dram_tensor("qT", [E, SQ], dt.bfloat16, kind="ExternalInput").ap()
    kT = nc.dram_tensor("kT", [E, skv], dt.bfloat16, kind="ExternalInput").ap()
    vT = nc.dram_tensor("vT", [E, skv], dt.bfloat16, kind="ExternalInput").ap()
    wqT = nc.dram_tensor("wqT", [E, DHC], dt.bfloat16, kind="ExternalInput").ap()
    wkT = nc.dram_tensor("wkT", [E, DHC], dt.bfloat16, kind="ExternalInput").ap()
    wvT = nc.dram_tensor("wvT", [E, DHC], dt.bfloat16, kind="ExternalInput").ap()
    woT = nc.dram_tensor("woT", [DHC, E], dt.bfloat16, kind="ExternalInput").ap()
    mb = nc.dram_tensor("mb", [128, nkv], dt.float32, kind="ExternalInput").ap()
    # bf16 partials: halves the output DMA; host sums the two partials in
    # fp32 (+bo), adding only ~0.3% rel err against a 2% gate
    outT = nc.dram_tensor("outT", [E, SQ], dt.bfloat16, kind="ExternalOutput").ap()

    ts = bass.ts
    kvchunks = _chunks512(skv)

    # 3D views of the DRAM inputs: (p, chunk, cols) with row 128*chunk+p
    qT3 = qT.rearrange("(e p) q -> p e q", p=128)
    kT3 = kT.rearrange("(e p) s -> p e s", p=128)
    vT3 = vT.rearrange("(e p) s -> p e s", p=128)
    wqT3 = wqT.rearrange("(e p) d -> p e d", p=128)
    wkT3 = wkT.rearrange("(e p) d -> p e d", p=128)
    wvT3 = wvT.rearrange("(e p) d -> p e d", p=128)
    woT3 = woT.rearrange("(c p) x -> p c x", p=128)

    with tile.TileContext(nc) as tc:
        pp = tc.alloc_tile_pool(name="persist", bufs=1)

        # Persistent SBUF tensors (merged per-tensor tiles; per-chunk views)
        wq_all = pp.tile([128, NE * DHC], dt.bfloat16, name="wqa", tag="wqa")
        wk_all = pp.tile([128, NE * DHC], dt.bfloat16, name="wka", tag="wka")
        wv_all = pp.tile([128, NE * DHC], dt.bfloat16, name="wva", tag="wva")
        wo_all = pp.tile([128, NDH * E], dt.bfloat16, name="woa", tag="woa")
        wq_sb = [wq_all[:, e * DHC:(e + 1) * DHC] for e in range(NE)]
        wk_sb = [wk_all[:, e * DHC:(e + 1) * DHC] for e in range(NE)]
        wv_sb = [wv_all[:, e * DHC:(e + 1) * DHC] for e in range(NE)]
        wo_sb = [wo_all[:, c * E:(c + 1) * E] for c in range(NDH)]
        # qh/aall split per q-half: separate tiles kill false WAR deps
        # between one half's reads and the other half's writes.
        qh_sb = [[pp.tile([128, SQH], dt.bfloat16, name=f"qh{h}_{c}", tag=f"qh{h}_{c}") for c in range(NDH)]
                 for h in range(2)]
        kh_sb = [pp.tile([128, skv], dt.bfloat16, name=f"kh{c}", tag=f"kh{c}") for c in range(NDH)]
        # V with per-head interleaved ones column: [kv, 8*(64+1)]
        va_sb = [pp.tile([128, HPC * (D + 1)], dt.bfloat16, name=f"va{j}", tag=f"va{j}") for j in range(nkv)]
        aall_sb = [[pp.tile([128, SQH], dt.bfloat16, name=f"aall{h}_{c}", tag=f"aall{h}_{c}") for c in range(NDH)]
                   for h in range(2)]
        mb_sb = pp.tile([128, nkv], dt.float32, name="mbt", tag="mbt")

        for j in range(nkv):
            nc.gpsimd.memset(va_sb[j][:, D::D + 1], 1.0)

        # warm-up operands: dependency-free matmuls during the initial DMA
        # wait keep HAM off the 1.2GHz cold clock until the lead units run
        wrm_w = pp.tile([128, 65], dt.bfloat16, name="wrmw", tag="wrmw")
        wrm_x = pp.tile([128, 512], dt.bfloat16, name="wrmx", tag="wrmx")
        wrm_e = pp.tile([1, 8], dt.bfloat16, name="wrme", tag="wrme")
        nc.gpsimd.memset(wrm_w[:], 0.0)
        nc.gpsimd.memset(wrm_x[:], 0.0)
        # dummy exp loads the ACT table set (~2.7us) during the DMA wait
        # instead of on the first real softmax exp
        nc.scalar.activation(
            wrm_e[:], wrm_x[0:1, 0:8], mybir.ActivationFunctionType.Exp,
        )

        # Input pools (released as the projections complete; right-side
        # stack so mid-stream release doesn't violate LIFO pool order)
        qip = tc.alloc_tile_pool(name="qinp", bufs=1, side="right")
        kip = tc.alloc_tile_pool(name="kinp", bufs=1, side="right")
        vip = tc.alloc_tile_pool(name="vinp", bufs=1, side="right")

        # P tiles: per slot, 2 heads x nkv chunks of [128, SQH] bf16
        ppool = tc.alloc_tile_pool(name="ppool", bufs=2)

        # PSUM pools: prj 2 banks + scores 4 banks + AV 2 banks = 8
        prj = tc.alloc_tile_pool(name="prj", bufs=2, space="PSUM")
        scp = tc.alloc_tile_pool(name="scp", bufs=1, space="PSUM")
        avp = tc.alloc_tile_pool(name="avp", bufs=2, space="PSUM")

        npool = [None]   # allocated after vip release
        opool = [None]   # allocated after qip/kip release

        # ---------------- input DMAs (fat descriptors, critical first) ----
        q1_all = qip.tile([128, NE * SQH], dt.bfloat16, name="qall", tag="qall")
        k_all = kip.tile([128, NE * skv], dt.bfloat16, name="kall", tag="kall")
        v_all = vip.tile([128, NE * skv], dt.bfloat16, name="vall", tag="vall")
        q1_sb = [q1_all[:, e * SQH:(e + 1) * SQH] for e in range(NE)]
        k_sb = [k_all[:, e * skv:(e + 1) * skv] for e in range(NE)]
        v_sb = [v_all[:, e * skv:(e + 1) * skv] for e in range(NE)]

        k3 = k_all.rearrange("p (e s) -> p e s", s=skv)
        v3 = v_all.rearrange("p (e s) -> p e s", s=skv)
        q13 = q1_all.rearrange("p (e q) -> p e q", q=SQH)

        kcut = min(512, skv)
        nc.sync.dma_start(wk_all[:], wkT3)
        nc.sync.dma_start(k3[:, :, 0:kcut], kT3[:, :, 0:kcut])
        nc.sync.dma_start(mb_sb[:], mb[:])
        nc.sync.dma_start(wq_all[:], wqT3)
        nc.sync.dma_start(q13[:, :, 0:512], qT3[:, :, 0:512])
        nc.sync.dma_start(q13[:, :, 512:SQH], qT3[:, :, 512:SQH])
        if skv > kcut:
            nc.sync.dma_start(k3[:, :, kcut:skv], kT3[:, :, kcut:skv])
        nc.sync.dma_start(wv_all[:], wvT3)
        # v in kv-thirds so early vproj units are not gated on the tail
        vth = max(128, ((nkv + 2) // 3) * 128)
        vcuts = sorted(set([min(vth, skv), min(2 * vth, skv), skv]))
        vprev = 0
        for vc in vcuts:
            if vc > vprev:
                nc.sync.dma_start(v3[:, :, vprev:vc], vT3[:, :, vprev:vc])
                vprev = vc
        nc.sync.dma_start(wo_all[:], woT3)

        # ---------------- projection / out-proj unit emitters ----------------
        # proj units either rotate through the prj pool (ptag None) or write
        # an explicitly provided psum slice (borrowed score banks).
        def _proj_ps(ptag):
            if ptag is None:
                return prj.tile([128, 512], dt.float32, name="pps", tag="prj")
            return ptag()

        def kproj_unit(c, o, w, ptag=None):
            def emit():
                kps = _proj_ps(ptag)
                for e in range(NE):
                    nc.tensor.matmul(
                        kps[:, 0:w], wk_sb[e][:, ts(c, 128)], k_sb[e][:, o:o + w],
                        start=(e == 0), stop=(e == NE - 1),
                    )
                nc.vector.tensor_copy(kh_sb[c][:, o:o + w], kps[:, 0:w])
            return emit

        def qproj_unit(c, half, t, q_tiles, ptag=None):
            def emit():
                qps = _proj_ps(ptag)
                for e in range(NE):
                    nc.tensor.matmul(
                        qps[:], wq_sb[e][:, ts(c, 128)], q_tiles[e][:, ts(t, 512)],
                        start=(e == 0), stop=(e == NE - 1),
                    )
                nc.vector.tensor_copy(qh_sb[half][c][:, ts(t, 512)], qps[:])
            return emit

        def vproj_unit(j):
            def emit():
                vps = prj.tile([128, DHC], dt.float32, name="vps", tag="prj")
                for e in range(NE):
                    nc.tensor.matmul(
                        vps[:], v_sb[e][:, ts(j, 128)], wv_sb[e][:],
                        start=(e == 0), stop=(e == NE - 1),
                    )
                dst = va_sb[j].rearrange("p (h x) -> p h x", x=D + 1)[:, :, 0:D]
                src = vps.rearrange("p (h x) -> p h x", x=D)
                nc.vector.tensor_copy(dst, src)
            return emit

        q2_holder = {}

        def qdma2_unit():
            def emit():
                q2_all = qip.tile([128, NE * SQH], dt.bfloat16, name="qall", tag="qall")
                q23 = q2_all.rearrange("p (e q) -> p e q", q=SQH)
                nc.sync.dma_start(q23[:, :, :], qT3[:, :, SQH:SQ])
                q2_holder["t"] = [q2_all[:, e * SQH:(e + 1) * SQH] for e in range(NE)]
            return emit

        def qproj2_unit(c, t):
            def emit():
                qps = prj.tile([128, 512], dt.float32, name="qps", tag="prj")
                for e in range(NE):
                    nc.tensor.matmul(
                        qps[:], wq_sb[e][:, ts(c, 128)], q2_holder["t"][e][:, ts(t, 512)],
                        start=(e == 0), stop=(e == NE - 1),
                    )
                nc.vector.tensor_copy(qh_sb[1][c][:, ts(t, 512)], qps[:])
            return emit

        def oproj_unit(half, eo, t, ptag=None, scalar_cp=False):
            def emit():
                ops = _proj_ps(ptag)
                for c in range(NDH):
                    nc.tensor.matmul(
                        ops[:], wo_sb[c][:, ts(eo, 128)],
                        aall_sb[half][c][:, ts(t, 512)],
                        start=(c == 0), stop=(c == NDH - 1),
                    )
                ob = opool[0].tile([128, 512], dt.bfloat16, name="ob", tag="ob", bufs=8)
                if scalar_cp:
                    # drain-only: ScalarE is idle after the last exp, so
                    # alternating psum evacuation between DVE and ScalarE
                    # halves the copy-chain latency gating each next unit
                    nc.scalar.copy(ob[:], ops[:])
                else:
                    nc.vector.tensor_copy(ob[:], ops[:])
                nc.sync.dma_start(
                    outT[ts(eo, 128), half * SQH + t * 512:half * SQH + (t + 1) * 512], ob[:])
            return emit

        # ---------------- normalization (per q-quarter, split chain) -------
        # pending_muls: (emit_fn, ready_step) — flushed >=2 j-steps after the
        # chain head so the DVE multiply never waits inside its queue.
        pending_muls = []
        step_ctr = [0]

        def norm_pre(hp, half, t, a2):
            c, r = hp // 2, hp % 2
            np_ = npool[0]
            au = np_.tile([D + 1, 512], dt.float32, name="au", tag="au", bufs=3)
            nc.vector.tensor_copy(au[:], a2[:])
            # spread Z across partitions; reciprocal cost is free-dim-size
            # bound.  1/Z flows back in bf16 (<=0.2% rel err) to halve the
            # rzr/rb SBUF footprint.
            zt = np_.tile([128, 4], dt.float32, name="zt", tag="zt", bufs=4)
            nc.sync.dma_start(zt[:], au[D:D + 1, :])
            rz8 = np_.tile([128, 4], dt.bfloat16, name="rz8", tag="rz8", bufs=4)
            with nc.allow_low_precision(reason="1/Z in bf16: <=0.2% rel err vs 2% gate"):
                nc.vector.reciprocal(rz8[:], zt[:])
            rzr = np_.tile([1, 512], dt.bfloat16, name="rzr", tag="rzr", bufs=4)
            nc.sync.dma_start(rzr[:], rz8[:])
            rb = np_.tile([D, 512], dt.bfloat16, name="rb", tag="rb", bufs=4)
            nc.gpsimd.partition_broadcast(rb[:], rzr[:])

            def mul():
                # mul on DVE, not gpsimd: keeping Pool broadcast-only avoids
                # the ~8us Q7 library reload between kernels.
                nc.vector.tensor_mul(
                    aall_sb[half][c][r * 64:(r + 1) * 64, ts(t, 512)],
                    au[0:D, :], rb[:],
                )
            pending_muls.append((mul, step_ctr[0] + 2))

        def flush_muls(all_=False):
            while pending_muls and (all_ or pending_muls[0][1] <= step_ctr[0]):
                pending_muls.pop(0)[0]()

        def flush_muls_n(n):
            for _ in range(min(n, len(pending_muls))):
                pending_muls.pop(0)[0]()

        # ---------------- AV machinery (t-major, 36 steps per slot) --------
        def av_step(state, k):
            ptAB, pair, half = state[0], state[1], state[2]
            local, rem = k // (2 * nkv), k % (2 * nkv)
            t, j = rem // nkv, rem % nkv
            hp = 2 * pair + local
            if j == 0:
                state[3][local * 2 + t] = avp.tile([D + 1, 512], dt.float32, name="a2", tag="a2")
            a2 = state[3][local * 2 + t]
            nc.tensor.matmul(
                a2[:],
                va_sb[j][:, hp * (D + 1):(hp + 1) * (D + 1)],
                ptAB[j][:, (2 * t + local) * 512:(2 * t + local + 1) * 512],
                start=(j == 0), stop=(j == nkv - 1),
            )
            if j == nkv - 1:
                norm_pre(hp, half, t, a2)

        # ---------------- filler schedule ----------------
        fillers = {s: [] for s in range(9)}
        # all vproj units live in slot 0 so the v input buffer is free (and
        # its space reusable for the norm pool) when slot 1 begins; the
        # k-tail-gated kproj(0) chunks also spill here so the lead-in's PE
        # FIFO never blocks on the k-tail DMA
        fillers[0] = [kproj_unit(1, *kvchunks[0]), qproj_unit(1, 0, 0, q1_sb)]
        fillers[0] += [kproj_unit(0, o, w) for (o, w) in kvchunks[1:]]
        fillers[0] += [qproj_unit(1, 0, 1, q1_sb)]
        fillers[0] += [kproj_unit(1, o, w) for (o, w) in kvchunks[1:]]
        fillers[0] += [vproj_unit(j) for j in range(nkv)]
        fillers[1] = [kproj_unit(2, o, w) for (o, w) in kvchunks]
        fillers[1] += [qproj_unit(2, 0, t, q1_sb) for t in range(2)]
        # qproj(3, H0) finishes the q1 reads in slot 1 so the q2 DMA (whose
        # buffer reuses q1's) can land mid-slot-2, before qproj2(0) needs it
        fillers[1] += [qproj_unit(3, 0, t, q1_sb) for t in range(2)]
        fillers[2] = [qdma2_unit()]
        fillers[2] += [kproj_unit(3, o, w) for (o, w) in kvchunks]
        fillers[2] += [qproj2_unit(0, t) for t in range(2)]
        fillers[3] = [qproj2_unit(1, t) for t in range(2)]
        fillers[3] += [qproj2_unit(2, t) for t in range(2)]
        fillers[4] = [qproj2_unit(3, t) for t in range(2)]
        oh0 = [oproj_unit(0, eo, t) for eo in range(NE) for t in range(2)]
        fillers[5] = oh0[0:6]
        fillers[6] = oh0[6:11]
        fillers[7] = oh0[11:16]

        def warm_mm():
            # dependency-free matmul into a dead PSUM tile (avp tag — free
            # during the drain, so no WAR pressure on the out-proj's score
            # banks): keeps the PE HAM-busy through cross-engine waits
            wps = avp.tile([D + 1, 512], dt.float32, name="a2", tag="a2")
            nc.tensor.matmul(
                wps[:], kh_sb[0][0:65, 0:D + 1], qh_sb[1][0][0:65, 0:512],
                start=True, stop=True,
            )

        # ---------------- lead-in: warmups, K(0), Q(0, H0) ----------------
        # dependency-free warmups bridge the fixed startup; the wk/k/q1-
        # gated ones fire as each DMA lands, so HAM coverage adapts to the
        # actual DMA pace and the lead projections always start at 2.4GHz.
        def _warm(lhs, rhs):
            wt = avp.tile([D + 1, 512], dt.float32, name="a2", tag="a2")
            nc.tensor.matmul(wt[:], lhs, rhs, start=True, stop=True)

        # FIFO order matters: the k-path (wk/k warmups, kproj ch0) goes
        # before anything q1-gated, so the first K-projection starts the
        # moment the k-head DMA lands instead of queueing behind warmups
        # that wait on the later q1 descriptors.
        for i in range(32):
            _warm(wrm_w[:], wrm_x[:])
        for i in range(8):
            _warm(wk_all[:, 0:D + 1], wrm_x[:])
        for i in range(6):
            _warm(wrm_w[:], k_sb[0][:, 0:512])
        # Q units borrow the score banks so none of the lead units
        # serialize on prj-pool rotation
        sl0 = scp.tile([128, SQH], dt.float32, name="sct0", tag="sct0")
        sl1 = scp.tile([128, SQH], dt.float32, name="sct1", tag="sct1")
        kproj_unit(0, *kvchunks[0])()
        for i in range(6):
            _warm(wrm_w[:], q1_sb[0][:, 0:512])
        qproj_unit(0, 0, 0, q1_sb, ptag=lambda: sl0[:, 0:512])()
        for i in range(3):
            _warm(wrm_w[:], q1_sb[0][:, 512:1024])
        qproj_unit(0, 0, 1, q1_sb, ptag=lambda: sl1[:, 0:512])()

        # ---------------- slot loop ----------------
        prev_state = None
        for s in range(9):
            if s == 1:
                vip.release()
                npool[0] = tc.alloc_tile_pool(name="norm", bufs=1)
            if s == 4:
                kip.release()
                qip.release()
                opool[0] = tc.alloc_tile_pool(name="outp", bufs=1)

            fl = list(fillers[s])
            n_emitted = 0

            if s < 8:
                half, pair = s // 4, s % 4
                # double-buffered fused P tiles (head A cols 0:1024, head B
                # cols 1024:2048): exp(s, j) only waits the PREVIOUS slot's
                # AV consumption, never same-slot PE progress
                ptAB = [ppool.tile([128, 2 * SQH], dt.bfloat16, name=f"pt{j}", tag=f"pt{j}", bufs=2) for j in range(nkv)]
                cur_state = [ptAB, pair, half, [None] * 4]

                for j in range(nkv):
                    step_ctr[0] += 1
                    # per-t score tiles, each [A-t|B-t]: one N=1024 exp per
                    # tile covers BOTH heads, so the A/B banks free at the
                    # same instant and the next j's A/B score matmuls
                    # co-issue (row-tiled, concurrent in the PE array);
                    # separate tags let scores(j+1,t0) run under exp(j,t1).
                    for t in range(2):
                        sct = scp.tile([128, SQH], dt.float32, name=f"sct{t}", tag=f"sct{t}")
                        nc.tensor.matmul(
                            sct[:, 0:512],
                            kh_sb[pair][0:64, ts(j, 128)],
                            qh_sb[half][pair][0:64, ts(t, 512)],
                            start=True, stop=True,
                        )
                        nc.tensor.matmul(
                            sct[:, 512:1024],
                            kh_sb[pair][64:128, ts(j, 128)],
                            qh_sb[half][pair][64:128, ts(t, 512)],
                            start=True, stop=True,
                        )
                        nc.scalar.activation(
                            ptAB[j][:, t * SQH:(t + 1) * SQH], sct[:],
                            mybir.ActivationFunctionType.Exp,
                            bias=mb_sb[:, j:j + 1], scale=SCALE,
                        )
                    if prev_state is not None:
                        for q in range(4):
                            av_step(prev_state, 4 * j + q)
                    flush_muls()
                    # spread filler units across the slot's chunks
                    want = (j + 1) * len(fl) // nkv
                    while n_emitted < want:
                        fl[n_emitted]()
                        n_emitted += 1
                prev_state = cur_state
            else:
                # drain slot: AV for slot 7 (A-t0, A-t1, B-t0, B-t1), with
                # the half-1 out-proj t=0 units hiding the B-t1 norm chain.
                # Muls are flushed when their broadcast chain is ready so
                # the Vector queue never blocks a later chain's head.
                # Units rotate over the 4 free score banks + the prj pool.
                grp = [None, None]

                def _drain_ptag(r):
                    def get():
                        if r == 0:
                            grp[0] = scp.tile([128, SQH], dt.float32, name="sct0", tag="sct0")
                            grp[1] = scp.tile([128, SQH], dt.float32, name="sct1", tag="sct1")
                        return grp[r // 2][:, (r % 2) * 512:(r % 2) * 512 + 512]
                    return get

                oh1 = []
                for u, (t, eo) in enumerate([(t, eo) for t in range(2) for eo in range(NE)]):
                    r = u % 6
                    oh1.append(oproj_unit(1, eo, t, ptag=_drain_ptag(r) if r < 4 else None,
                                          scalar_cp=(u % 2 == 1)))
                for k in range(3 * nkv):
                    av_step(prev_state, k)
                    if k == nkv - 1:
                        flush_muls_n(1)         # slot-6 B-t1 leftover
                flush_muls_n(1)                 # A-t0 (2 sweeps old)
                for k in range(3 * nkv, 4 * nkv):
                    av_step(prev_state, k)
                flush_muls_n(1)                 # A-t1 (chain long done)
                for _ in range(6):
                    warm_mm()
                flush_muls_n(1)                 # B-t0 (~ready after the warms)
                for _ in range(6):
                    warm_mm()
                for f in oh1[0:NE]:
                    f()
                flush_muls(all_=True)           # B-t1 (hidden behind 8 units)
                for f in oh1[NE:]:
                    f()

        for pool in (opool[0], npool[0], avp, scp, prj, ppool, pp):
            pool.release()

    nc.compile()
    _PROGRAM_CACHE[skv] = nc
    return nc


def make_in_maps(q, k, v, mask, Wq, Wk, Wv, Wo, skv):
    """Host-side shard/compact/transpose/cast. Returns per-core input dicts."""
    in_maps = []
    valid = mask != 0
    for core in range(N_CORES):
        b, hg = core // 2, core % 2
        idx = np.nonzero(valid[b])[0]
        cnt = len(idx)

        kc = np.zeros((skv, E), np.float32)
        vc = np.zeros((skv, E), np.float32)
        kc[:cnt] = k[b][idx]
        vc[:cnt] = v[b][idx]

        mbias = np.zeros((skv,), np.float32)
        mbias[cnt:] = NEG
        # [128, nkv]: column j = kv chunk j
        mb2 = np.ascontiguousarray(mbias.reshape(-1, 128).T)

        rows = slice(hg * DHC, (hg + 1) * DHC)
        in_maps.append(dict(
            qT=np.ascontiguousarray(q[b].T).astype(BF16),
            kT=np.ascontiguousarray(kc.T).astype(BF16),
            vT=np.ascontiguousarray(vc.T).astype(BF16),
            wqT=np.ascontiguousarray(Wq[rows, :].T).astype(BF16),
            wkT=np.ascontiguousarray(Wk[rows, :].T).astype(BF16),
            wvT=np.ascontiguousarray(Wv[rows, :].T).astype(BF16),
            woT=np.ascontiguousarray(Wo[:, rows].T).astype(BF16),
            mb=mb2,
        ))
    return in_maps


def _numpy_fallback(q, k, v, mask, Wq, bq, Wk, bk, Wv, bv, Wo, bo):
    out = np.zeros((B, SQ, E), np.float32)
    for b in range(B):
        qh = (q[b] @ Wq.T + bq).reshape(SQ, H_TOT, D).transpose(1, 0, 2)
        kh = (k[b] @ Wk.T + bk).reshape(-1, H_TOT, D).transpose(1, 0, 2)
        vh = (v[b] @ Wv.T + bv).reshape(-1, H_TOT, D).transpose(1, 0, 2)
        att = np.einsum("hqd,hkd->hqk", qh, kh) * SCALE
        valid = mask[b] != 0
        if not valid.any():
            out[b] = bo
            continue
        att = np.where(valid[None, None, :], att, -np.inf)
        att = att - att.max(-1, keepdims=True)
        att = np.exp(att)
        att /= att.sum(-1, keepdims=True)
        o = np.einsum("hqk,hkd->hqd", att, vh)
        o = o.transpose(1, 0, 2).reshape(SQ, E)
        out[b] = o @ Wo.T + bo
    return out


def kernel(**inputs):
    global LAST_RESULTS
    q = np.asarray(inputs["q"], np.float32)
    k = np.asarray(inputs["k"], np.float32)
    v = np.asarray(inputs["v"], np.float32)
    mask = np.asarray(inputs["mask"])
    Wq, bq = np.asarray(inputs["Wq"], np.float32), np.asarray(inputs["bq"], np.float32)
    Wk, bk = np.asarray(inputs["Wk"], np.float32), np.asarray(inputs["bk"], np.float32)
    Wv, bv = np.asarray(inputs["Wv"], np.float32), np.asarray(inputs["bv"], np.float32)
    Wo, bo = np.asarray(inputs["Wo"], np.float32), np.asarray(inputs["bo"], np.float32)

    if any(np.abs(x).max() > 0 for x in (bq, bk, bv)):
        # q/k/v biases are zero in this problem's setup; a nonzero bias
        # would need the augmented-contraction path, so fall back.
        return _numpy_fallback(q, k, v, mask, Wq, bq, Wk, bk, Wv, bv, Wo, bo)

    valid = mask != 0
    counts = valid.sum(axis=1)
    if counts.max() == 0:
        return np.broadcast_to(bo, (B, SQ, E)).astype(np.float32).copy()

    skv = int(-(-counts.max() // 128) * 128)
    nc = build_program(skv)
    in_maps = make_in_maps(q, k, v, mask, Wq, Wk, Wv, Wo, skv)

    res = bass_utils.run_bass_kernel_spmd(nc, in_maps, core_ids=list(range(N_CORES)))
    LAST_RESULTS = res

    out = np.empty((B, SQ, E), np.float32)
    for b in range(B):
        if counts[b] == 0:
            out[b] = bo
        else:
            p0 = np.asarray(res.results[2 * b]["outT"], np.float32)
            p1 = np.asarray(res.results[2 * b + 1]["outT"], np.float32)
            out[b] = p0.T + p1.T + bo
    return out


# revision 36
# speedup vs baseline: 1.0182x; 1.0107x over previous
"""Multi-head attention (batched, key-padding mask) Trainium2 Bass kernel — v3.

Problem: nn_MultiHeadBatched
  q,k,v: [B=4, S=2048, E=1024] fp32; mask: [B, 2048] int32 (key padding)
  16 heads, head_dim 64; torch-Linear style q/k/v/out projections.

Sharding (8 cores): core c handles batch b=c//2 and head group hg=c%2
(8 heads each).  q/k/v projections are column-parallel over the head
group; out-projection is row-parallel — each core produces a partial
[E, Sq] output and the host sums the two partials per batch (+ bo).

v3+ changes over v2 (291us -> ~251us measured):
  - Fat DMA descriptors: each input tensor loads via 1-3 multi-dim
    descriptors instead of per-chunk dma_starts.  Packets of one
    descriptor round-robin over all 16 DMA engines, so batching keeps
    full HBM bandwidth while cutting Sync-engine descriptor-issue time
    (607ns each) from ~32us to ~8us; critical-path tensors (wk, k-head,
    mb, wq, q1) are ordered first.
  - Per-t score tiles [A-t|B-t] with one N=1024 exp each: both heads'
    score banks free at the same instant, so the next chunk's A/B score
    matmuls become ready together and co-issue row-tiled (concurrent in
    the PE array, 2x score throughput — with separate scA/scB tiles the
    staggered exp reads serialized the pairs); the two t-tags pipeline
    against each other so the PE never waits on the ACT engine.
  - t-major AV with per-t normalization: the AV accumulators are
    [65, 512] per (head, q-quarter); norm chains are half as long and
    the final-slot chain is hidden behind the half-1 out-proj t=0 units.
  - Deferred norm multiplies: the 1/Z broadcast chain (DVE copy -> DMA
    spread -> reciprocal -> DMA back -> Pool broadcast) is emitted at AV
    end, but the closing DVE multiply is queued and flushed ~2 j-steps
    later, so the Vector engine never head-of-line blocks on the chain
    (that stall starved the PE for 2-6us at slot boundaries in v2).
  - Lead-in warmup matmuls (free-running plus wk/k/q1-DMA-gated ones
    that self-pace with the actual DMA arrival) and drain warmups keep
    the PE HAM un-throttled (2.4GHz) for the entire kernel.
"""

import os
import sys

import numpy as np

sys.path.insert(0, "/opt/trn_rl_repo")

import concourse.bass as bass
import concourse.bacc as bacc
import concourse.mybir as mybir
import concourse.tile as tile
from concourse import bass_utils

import ml_dtypes

BF16 = ml_dtypes.bfloat16

B, SQ, E = 4, 2048, 1024
H_TOT, D = 16, 64
HPC = H_TOT // 2            # heads per core (head-group split in 2)
DHC = HPC * D               # 512 projected channels per core
NE = E // 128               # contraction chunks
NDH = DHC // 128            # dh chunks per core
SQH = SQ // 2               # q-half width
NEG = -1.0e30
SCALE = D ** -0.5

N_CORES = 8

_PROGRAM_CACHE = {}
LAST_RESULTS = None


def _chunks512(n):
    out = []
    o = 0
    while o < n:
        w = min(512, n - o)
        out.append((o, w))
        o += w
    return out


def build_program(skv):
    """Build + compile the single-core SPMD Bass program for padded KV
    length `skv` (multiple of 128)."""
    if skv in _PROGRAM_CACHE:
        return _PROGRAM_CACHE[skv]

    nkv = skv // 128
    dt = mybir.dt

    nc = bacc.Bacc(
        "TRN2",
        target_bir_lowering=False,
        debug=False,
        enable_asserts=False,
        num_devices=N_CORES,
    )

    # DRAM I/O (per-core shapes)
    qT = nc.dram_tensor("qT", [E, SQ], dt.bfloat16, kind="ExternalInput").ap()
    kT = nc.dram_tensor("kT", [E, skv], dt.bfloat16, kind="ExternalInput").ap()
    vT = nc.dram_tensor("vT", [E, skv], dt.bfloat16, kind="ExternalInput").ap()
    wqT = nc.dram_tensor("wqT", [E, DHC], dt.bfloat16, kind="ExternalInput").ap()
    wkT = nc.dram_tensor("wkT", [E, DHC], dt.bfloat16, kind="ExternalInput").ap()
    wvT = nc.dram_tensor("wvT", [E, DHC], dt.bfloat16, kind="ExternalInput").ap()
    woT = nc.dram_tensor("woT", [DHC, E], dt.bfloat16, kind="ExternalInput").ap()
    mb = nc.dram_tensor("mb", [128, nkv], dt.float32, kind="ExternalInput").ap()
    # bf16 partials: halves the output DMA; host sums the two partials in
    # fp32 (+bo), adding only ~0.3% rel err against a 2% gate
    outT = nc.dram_tensor("outT", [E, SQ], dt.bfloat16, kind="ExternalOutput").ap()

    ts = bass.ts
    kvchunks = _chunks512(skv)

    # 3D views of the DRAM inputs: (p, chunk, cols) with row 128*chunk+p
    qT3 = qT.rearrange("(e p) q -> p e q", p=128)
    kT3 = kT.rearrange("(e p) s -> p e s", p=128)
    vT3 = vT.rearrange("(e p) s -> p e s", p=128)
    wqT3 = wqT.rearrange("(e p) d -> p e d", p=128)
    wkT3 = wkT.rearrange("(e p) d -> p e d", p=128)
    wvT3 = wvT.rearrange("(e p) d -> p e d", p=128)
    woT3 = woT.rearrange("(c p) x -> p c x", p=128)

    with tile.TileContext(nc) as tc:
        pp = tc.alloc_tile_pool(name="persist", bufs=1)

        # Persistent SBUF tensors (merged per-tensor tiles; per-chunk views)
        wq_all = pp.tile([128, NE * DHC], dt.bfloat16, name="wqa", tag="wqa")
        wk_all = pp.tile([128, NE * DHC], dt.bfloat16, name="wka", tag="wka")
        wv_all = pp.tile([128, NE * DHC], dt.bfloat16, name="wva", tag="wva")
        wo_all = pp.tile([128, NDH * E], dt.bfloat16, name="woa", tag="woa")
        wq_sb = [wq_all[:, e * DHC:(e + 1) * DHC] for e in range(NE)]
        wk_sb = [wk_all[:, e * DHC:(e + 1) * DHC] for e in range(NE)]
        wv_sb = [wv_all[:, e * DHC:(e + 1) * DHC] for e in range(NE)]
        wo_sb = [wo_all[:, c * E:(c + 1) * E] for c in range(NDH)]
        # qh/aall split per q-half: separate tiles kill false WAR deps
        # between one half's reads and the other half's writes.
        qh_sb = [[pp.tile([128, SQH], dt.bfloat16, name=f"qh{h}_{c}", tag=f"qh{h}_{c}") for c in range(NDH)]
                 for h in range(2)]
        kh_sb = [pp.tile([128, skv], dt.bfloat16, name=f"kh{c}", tag=f"kh{c}") for c in range(NDH)]
        # V with per-head interleaved ones column: [kv, 8*(64+1)]
        va_sb = [pp.tile([128, HPC * (D + 1)], dt.bfloat16, name=f"va{j}", tag=f"va{j}") for j in range(nkv)]
        aall_sb = [[pp.tile([128, SQH], dt.bfloat16, name=f"aall{h}_{c}", tag=f"aall{h}_{c}") for c in range(NDH)]
                   for h in range(2)]
        mb_sb = pp.tile([128, nkv], dt.float32, name="mbt", tag="mbt")

        for j in range(nkv):
            nc.gpsimd.memset(va_sb[j][:, D::D + 1], 1.0)

        # warm-up operands: dependency-free matmuls during the initial DMA
        # wait keep HAM off the 1.2GHz cold clock until the lead units run
        wrm_w = pp.tile([128, 65], dt.bfloat16, name="wrmw", tag="wrmw")
        wrm_x = pp.tile([128, 512], dt.bfloat16, name="wrmx", tag="wrmx")
        wrm_e = pp.tile([1, 8], dt.bfloat16, name="wrme", tag="wrme")
        nc.gpsimd.memset(wrm_w[:], 0.0)
        nc.gpsimd.memset(wrm_x[:], 0.0)
        # dummy exp loads the ACT table set (~2.7us) during the DMA wait
        # instead of on the first real softmax exp
        nc.scalar.activation(
            wrm_e[:], wrm_x[0:1, 0:8], mybir.ActivationFunctionType.Exp,
        )

        # Input pools (released as the projections complete; right-side
        # stack so mid-stream release doesn't violate LIFO pool order)
        qip = tc.alloc_tile_pool(name="qinp", bufs=1, side="right")
        kip = tc.alloc_tile_pool(name="kinp", bufs=1, side="right")
        vip = tc.alloc_tile_pool(name="vinp", bufs=1, side="right")

        # P tiles: per slot, 2 heads x nkv chunks of [128, SQH] bf16
        ppool = tc.alloc_tile_pool(name="ppool", bufs=2)

        # PSUM pools: prj 2 banks + scores 4 banks + AV 2 banks = 8
        prj = tc.alloc_tile_pool(name="prj", bufs=2, space="PSUM")
        scp = tc.alloc_tile_pool(name="scp", bufs=1, space="PSUM")
        avp = tc.alloc_tile_pool(name="avp", bufs=2, space="PSUM")

        npool = [None]   # allocated after vip release
        opool = [None]   # allocated after qip/kip release

        # ---------------- input DMAs (fat descriptors, critical first) ----
        q1_all = qip.tile([128, NE * SQH], dt.bfloat16, name="qall", tag="qall")
        k_all = kip.tile([128, NE * skv], dt.bfloat16, name="kall", tag="kall")
        v_all = vip.tile([128, NE * skv], dt.bfloat16, name="vall", tag="vall")
        q1_sb = [q1_all[:, e * SQH:(e + 1) * SQH] for e in range(NE)]
        k_sb = [k_all[:, e * skv:(e + 1) * skv] for e in range(NE)]
        v_sb = [v_all[:, e * skv:(e + 1) * skv] for e in range(NE)]

        k3 = k_all.rearrange("p (e s) -> p e s", s=skv)
        v3 = v_all.rearrange("p (e s) -> p e s", s=skv)
        q13 = q1_all.rearrange("p (e q) -> p e q", q=SQH)

        kcut = min(512, skv)
        nc.sync.dma_start(wk_all[:], wkT3)
        nc.sync.dma_start(k3[:, :, 0:kcut], kT3[:, :, 0:kcut])
        nc.sync.dma_start(mb_sb[:], mb[:])
        nc.sync.dma_start(wq_all[:], wqT3)
        nc.sync.dma_start(q13[:, :, 0:512], qT3[:, :, 0:512])
        nc.sync.dma_start(q13[:, :, 512:SQH], qT3[:, :, 512:SQH])
        if skv > kcut:
            nc.sync.dma_start(k3[:, :, kcut:skv], kT3[:, :, kcut:skv])
        nc.sync.dma_start(wv_all[:], wvT3)
        # v in kv-thirds so early vproj units are not gated on the tail
        vth = max(128, ((nkv + 2) // 3) * 128)
        vcuts = sorted(set([min(vth, skv), min(2 * vth, skv), skv]))
        vprev = 0
        for vc in vcuts:
            if vc > vprev:
                nc.sync.dma_start(v3[:, :, vprev:vc], vT3[:, :, vprev:vc])
                vprev = vc
        nc.sync.dma_start(wo_all[:], woT3)

        # ---------------- projection / out-proj unit emitters ----------------
        # proj units either rotate through the prj pool (ptag None) or write
        # an explicitly provided psum slice (borrowed score banks).
        def _proj_ps(ptag):
            if ptag is None:
                return prj.tile([128, 512], dt.float32, name="pps", tag="prj")
            return ptag()

        def kproj_unit(c, o, w, ptag=None):
            def emit():
                kps = _proj_ps(ptag)
                for e in range(NE):
                    nc.tensor.matmul(
                        kps[:, 0:w], wk_sb[e][:, ts(c, 128)], k_sb[e][:, o:o + w],
                        start=(e == 0), stop=(e == NE - 1),
                    )
                nc.vector.tensor_copy(kh_sb[c][:, o:o + w], kps[:, 0:w])
            return emit

        def qproj_unit(c, half, t, q_tiles, ptag=None):
            def emit():
                qps = _proj_ps(ptag)
                for e in range(NE):
                    nc.tensor.matmul(
                        qps[:], wq_sb[e][:, ts(c, 128)], q_tiles[e][:, ts(t, 512)],
                        start=(e == 0), stop=(e == NE - 1),
                    )
                nc.vector.tensor_copy(qh_sb[half][c][:, ts(t, 512)], qps[:])
            return emit

        def vproj_unit(j):
            def emit():
                vps = prj.tile([128, DHC], dt.float32, name="vps", tag="prj")
                for e in range(NE):
                    nc.tensor.matmul(
                        vps[:], v_sb[e][:, ts(j, 128)], wv_sb[e][:],
                        start=(e == 0), stop=(e == NE - 1),
                    )
                dst = va_sb[j].rearrange("p (h x) -> p h x", x=D + 1)[:, :, 0:D]
                src = vps.rearrange("p (h x) -> p h x", x=D)
                nc.vector.tensor_copy(dst, src)
            return emit

        q2_holder = {}

        def qdma2_unit():
            def emit():
                q2_all = qip.tile([128, NE * SQH], dt.bfloat16, name="qall", tag="qall")
                q23 = q2_all.rearrange("p (e q) -> p e q", q=SQH)
                nc.sync.dma_start(q23[:, :, :], qT3[:, :, SQH:SQ])
                q2_holder["t"] = [q2_all[:, e * SQH:(e + 1) * SQH] for e in range(NE)]
            return emit

        def qproj2_unit(c, t):
            def emit():
                qps = prj.tile([128, 512], dt.float32, name="qps", tag="prj")
                for e in range(NE):
                    nc.tensor.matmul(
                        qps[:], wq_sb[e][:, ts(c, 128)], q2_holder["t"][e][:, ts(t, 512)],
                        start=(e == 0), stop=(e == NE - 1),
                    )
                nc.vector.tensor_copy(qh_sb[1][c][:, ts(t, 512)], qps[:])
            return emit

        def oproj_unit(half, eo, t, ptag=None, scalar_cp=False):
            def emit():
                ops = _proj_ps(ptag)
                for c in range(NDH):
                    nc.tensor.matmul(
                        ops[:], wo_sb[c][:, ts(eo, 128)],
                        aall_sb[half][c][:, ts(t, 512)],
                        start=(c == 0), stop=(c == NDH - 1),
                    )
                ob = opool[0].tile([128, 512], dt.bfloat16, name="ob", tag="ob", bufs=8)
                if scalar_cp:
                    # drain-only: ScalarE is idle after the last exp, so
                    # alternating psum evacuation between DVE and ScalarE
                    # halves the copy-chain latency gating each next unit
                    nc.scalar.copy(ob[:], ops[:])
                else:
                    nc.vector.tensor_copy(ob[:], ops[:])
                nc.sync.dma_start(
                    outT[ts(eo, 128), half * SQH + t * 512:half * SQH + (t + 1) * 512], ob[:])
            return emit

        # ---------------- normalization (per q-quarter, split chain) -------
        # pending_muls: (emit_fn, ready_step) — flushed >=2 j-steps after the
        # chain head so the DVE multiply never waits inside its queue.
        pending_muls = []
        step_ctr = [0]

        drain_mode = [False]

        def norm_pre(hp, half, t, a2):
            c, r = hp // 2, hp % 2
            np_ = npool[0]
            au = np_.tile([D + 1, 512], dt.float32, name="au", tag="au", bufs=3)
            if drain_mode[0]:
                # ScalarE is idle in the drain; keeping the au evacuation
                # off the Vector queue stops deferred-mul waits from
                # blocking the AV accumulator rotation
                nc.scalar.copy(au[:], a2[:])
            else:
                nc.vector.tensor_copy(au[:], a2[:])
            # spread Z across partitions; reciprocal cost is free-dim-size
            # bound.  1/Z flows back in bf16 (<=0.2% rel err) to halve the
            # rzr/rb SBUF footprint.
            zt = np_.tile([128, 4], dt.float32, name="zt", tag="zt", bufs=4)
            nc.sync.dma_start(zt[:], au[D:D + 1, :])
            rz8 = np_.tile([128, 4], dt.bfloat16, name="rz8", tag="rz8", bufs=4)
            with nc.allow_low_precision(reason="1/Z in bf16: <=0.2% rel err vs 2% gate"):
                nc.vector.reciprocal(rz8[:], zt[:])
            rzr = np_.tile([1, 512], dt.bfloat16, name="rzr", tag="rzr", bufs=4)
            nc.sync.dma_start(rzr[:], rz8[:])
            rb = np_.tile([D, 512], dt.bfloat16, name="rb", tag="rb", bufs=4)
            nc.gpsimd.partition_broadcast(rb[:], rzr[:])

            def mul():
                # mul on DVE, not gpsimd: keeping Pool broadcast-only avoids
                # the ~8us Q7 library reload between kernels.
                nc.vector.tensor_mul(
                    aall_sb[half][c][r * 64:(r + 1) * 64, ts(t, 512)],
                    au[0:D, :], rb[:],
                )
            pending_muls.append((mul, step_ctr[0] + 2))

        def flush_muls(all_=False):
            while pending_muls and (all_ or pending_muls[0][1] <= step_ctr[0]):
                pending_muls.pop(0)[0]()

        def flush_muls_n(n):
            for _ in range(min(n, len(pending_muls))):
                pending_muls.pop(0)[0]()

        # ---------------- AV machinery (t-major, 36 steps per slot) --------
        def av_step(state, k):
            ptAB, pair, half = state[0], state[1], state[2]
            local, rem = k // (2 * nkv), k % (2 * nkv)
            t, j = rem // nkv, rem % nkv
            hp = 2 * pair + local
            if j == 0:
                state[3][local * 2 + t] = avp.tile([D + 1, 512], dt.float32, name="a2", tag="a2")
            a2 = state[3][local * 2 + t]
            nc.tensor.matmul(
                a2[:],
                va_sb[j][:, hp * (D + 1):(hp + 1) * (D + 1)],
                ptAB[j][:, (2 * t + local) * 512:(2 * t + local + 1) * 512],
                start=(j == 0), stop=(j == nkv - 1),
            )
            if j == nkv - 1:
                norm_pre(hp, half, t, a2)

        # ---------------- filler schedule ----------------
        fillers = {s: [] for s in range(9)}
        # all vproj units live in slot 0 so the v input buffer is free (and
        # its space reusable for the norm pool) when slot 1 begins; the
        # k-tail-gated kproj(0) chunks also spill here so the lead-in's PE
        # FIFO never blocks on the k-tail DMA
        fillers[0] = [kproj_unit(1, *kvchunks[0]), qproj_unit(1, 0, 0, q1_sb)]
        fillers[0] += [kproj_unit(0, o, w) for (o, w) in kvchunks[1:]]
        fillers[0] += [qproj_unit(1, 0, 1, q1_sb)]
        fillers[0] += [kproj_unit(1, o, w) for (o, w) in kvchunks[1:]]
        fillers[0] += [vproj_unit(j) for j in range(nkv)]
        fillers[1] = [kproj_unit(2, o, w) for (o, w) in kvchunks]
        fillers[1] += [qproj_unit(2, 0, t, q1_sb) for t in range(2)]
        # qproj(3, H0) finishes the q1 reads in slot 1 so the q2 DMA (whose
        # buffer reuses q1's) can land mid-slot-2, before qproj2(0) needs it
        fillers[1] += [qproj_unit(3, 0, t, q1_sb) for t in range(2)]
        fillers[2] = [qdma2_unit()]
        fillers[2] += [kproj_unit(3, o, w) for (o, w) in kvchunks]
        fillers[2] += [qproj2_unit(0, t) for t in range(2)]
        fillers[3] = [qproj2_unit(1, t) for t in range(2)]
        fillers[3] += [qproj2_unit(2, t) for t in range(2)]
        fillers[4] = [qproj2_unit(3, t) for t in range(2)]
        oh0 = [oproj_unit(0, eo, t) for eo in range(NE) for t in range(2)]
        fillers[5] = oh0[0:6]
        fillers[6] = oh0[6:11]
        fillers[7] = oh0[11:16]

        def warm_mm():
            # dependency-free matmul into a dead PSUM tile (avp tag — free
            # during the drain, so no WAR pressure on the out-proj's score
            # banks): keeps the PE HAM-busy through cross-engine waits
            wps = avp.tile([D + 1, 512], dt.float32, name="a2", tag="a2")
            nc.tensor.matmul(
                wps[:], kh_sb[0][0:65, 0:D + 1], qh_sb[1][0][0:65, 0:512],
                start=True, stop=True,
            )

        # ---------------- lead-in: warmups, K(0), Q(0, H0) ----------------
        # dependency-free warmups bridge the fixed startup; the wk/k/q1-
        # gated ones fire as each DMA lands, so HAM coverage adapts to the
        # actual DMA pace and the lead projections always start at 2.4GHz.
        def _warm(lhs, rhs):
            wt = avp.tile([D + 1, 512], dt.float32, name="a2", tag="a2")
            nc.tensor.matmul(wt[:], lhs, rhs, start=True, stop=True)

        # FIFO order matters: the k-path (wk/k warmups, kproj ch0) goes
        # before anything q1-gated, so the first K-projection starts the
        # moment the k-head DMA lands instead of queueing behind warmups
        # that wait on the later q1 descriptors.
        for i in range(32):
            _warm(wrm_w[:], wrm_x[:])
        for i in range(8):
            _warm(wk_all[:, 0:D + 1], wrm_x[:])
        for i in range(6):
            _warm(wrm_w[:], k_sb[0][:, 0:512])
        # Q units borrow the score banks so none of the lead units
        # serialize on prj-pool rotation
        sl0 = scp.tile([128, SQH], dt.float32, name="sct0", tag="sct0")
        sl1 = scp.tile([128, SQH], dt.float32, name="sct1", tag="sct1")
        kproj_unit(0, *kvchunks[0])()
        for i in range(6):
            _warm(wrm_w[:], q1_sb[0][:, 0:512])
        qproj_unit(0, 0, 0, q1_sb, ptag=lambda: sl0[:, 0:512])()
        for i in range(3):
            _warm(wrm_w[:], q1_sb[0][:, 512:1024])
        qproj_unit(0, 0, 1, q1_sb, ptag=lambda: sl1[:, 0:512])()

        # ---------------- slot loop ----------------
        prev_state = None
        for s in range(9):
            if s == 1:
                vip.release()
                npool[0] = tc.alloc_tile_pool(name="norm", bufs=1)
            if s == 4:
                kip.release()
                qip.release()
                opool[0] = tc.alloc_tile_pool(name="outp", bufs=1)

            fl = list(fillers[s])
            n_emitted = 0

            if s < 8:
                half, pair = s // 4, s % 4
                # double-buffered fused P tiles (head A cols 0:1024, head B
                # cols 1024:2048): exp(s, j) only waits the PREVIOUS slot's
                # AV consumption, never same-slot PE progress
                ptAB = [ppool.tile([128, 2 * SQH], dt.bfloat16, name=f"pt{j}", tag=f"pt{j}", bufs=2) for j in range(nkv)]
                cur_state = [ptAB, pair, half, [None] * 4]

                for j in range(nkv):
                    step_ctr[0] += 1
                    # per-t score tiles, each [A-t|B-t]: one N=1024 exp per
                    # tile covers BOTH heads, so the A/B banks free at the
                    # same instant and the next j's A/B score matmuls
                    # co-issue (row-tiled, concurrent in the PE array);
                    # separate tags let scores(j+1,t0) run under exp(j,t1).
                    for t in range(2):
                        sct = scp.tile([128, SQH], dt.float32, name=f"sct{t}", tag=f"sct{t}")
                        nc.tensor.matmul(
                            sct[:, 0:512],
                            kh_sb[pair][0:64, ts(j, 128)],
                            qh_sb[half][pair][0:64, ts(t, 512)],
                            start=True, stop=True,
                        )
                        nc.tensor.matmul(
                            sct[:, 512:1024],
                            kh_sb[pair][64:128, ts(j, 128)],
                            qh_sb[half][pair][64:128, ts(t, 512)],
                            start=True, stop=True,
                        )
                        nc.scalar.activation(
                            ptAB[j][:, t * SQH:(t + 1) * SQH], sct[:],
                            mybir.ActivationFunctionType.Exp,
                            bias=mb_sb[:, j:j + 1], scale=SCALE,
                        )
                    if prev_state is not None:
                        for q in range(4):
                            av_step(prev_state, 4 * j + q)
                    flush_muls()
                    # spread filler units across the slot's chunks
                    want = (j + 1) * len(fl) // nkv
                    while n_emitted < want:
                        fl[n_emitted]()
                        n_emitted += 1
                prev_state = cur_state
            else:
                # drain slot: AV for slot 7 (A-t0, A-t1, B-t0, B-t1), with
                # the half-1 out-proj t=0 units hiding the B-t1 norm chain.
                # Muls are flushed when their broadcast chain is ready so
                # the Vector queue never blocks a later chain's head.
                # Units rotate over the 4 free score banks + the prj pool.
                grp = [None, None]

                def _drain_ptag(r):
                    def get():
                        if r == 0:
                            grp[0] = scp.tile([128, SQH], dt.float32, name="sct0", tag="sct0")
                            grp[1] = scp.tile([128, SQH], dt.float32, name="sct1", tag="sct1")
                        return grp[r // 2][:, (r % 2) * 512:(r % 2) * 512 + 512]
                    return get

                oh1 = []
                for u, (t, eo) in enumerate([(t, eo) for t in range(2) for eo in range(NE)]):
                    r = u % 6
                    oh1.append(oproj_unit(1, eo, t, ptag=_drain_ptag(r) if r < 4 else None,
                                          scalar_cp=(u % 2 == 1)))
                drain_mode[0] = True
                for k in range(3 * nkv):
                    av_step(prev_state, k)
                    if k == 2 * nkv - 1:
                        flush_muls_n(1)         # slot-6 B-t1 leftover
                flush_muls_n(1)                 # A-t0 (2 sweeps old)
                for k in range(3 * nkv, 4 * nkv):
                    av_step(prev_state, k)
                flush_muls_n(1)                 # A-t1 (chain long done)
                for _ in range(6):
                    warm_mm()
                flush_muls_n(1)                 # B-t0 (~ready after the warms)
                for _ in range(6):
                    warm_mm()
                for f in oh1[0:NE]:
                    f()
                flush_muls(all_=True)           # B-t1 (hidden behind 8 units)
                for f in oh1[NE:]:
                    f()

        for pool in (opool[0], npool[0], avp, scp, prj, ppool, pp):
            pool.release()

    nc.compile()
    _PROGRAM_CACHE[skv] = nc
    return nc


def make_in_maps(q, k, v, mask, Wq, Wk, Wv, Wo, skv):
    """Host-side shard/compact/transpose/cast. Returns per-core input dicts."""
    in_maps = []
    valid = mask != 0
    for core in range(N_CORES):
        b, hg = core // 2, core % 2
        idx = np.nonzero(valid[b])[0]
        cnt = len(idx)

        kc = np.zeros((skv, E), np.float32)
        vc = np.zeros((skv, E), np.float32)
        kc[:cnt] = k[b][idx]
        vc[:cnt] = v[b][idx]

        mbias = np.zeros((skv,), np.float32)
        mbias[cnt:] = NEG
        # [128, nkv]: column j = kv chunk j
        mb2 = np.ascontiguousarray(mbias.reshape(-1, 128).T)

        rows = slice(hg * DHC, (hg + 1) * DHC)
        in_maps.append(dict(
            qT=np.ascontiguousarray(q[b].T).astype(BF16),
            kT=np.ascontiguousarray(kc.T).astype(BF16),
            vT=np.ascontiguousarray(vc.T).astype(BF16),
            wqT=np.ascontiguousarray(Wq[rows, :].T).astype(BF16),
            wkT=np.ascontiguousarray(Wk[rows, :].T).astype(BF16),
            wvT=np.ascontiguousarray(Wv[rows, :].T).astype(BF16),
            woT=np.ascontiguousarray(Wo[:, rows].T).astype(BF16),
            mb=mb2,
        ))
    return in_maps


def _numpy_fallback(q, k, v, mask, Wq, bq, Wk, bk, Wv, bv, Wo, bo):
    out = np.zeros((B, SQ, E), np.float32)
    for b in range(B):
        qh = (q[b] @ Wq.T + bq).reshape(SQ, H_TOT, D).transpose(1, 0, 2)
        kh = (k[b] @ Wk.T + bk).reshape(-1, H_TOT, D).transpose(1, 0, 2)
        vh = (v[b] @ Wv.T + bv).reshape(-1, H_TOT, D).transpose(1, 0, 2)
        att = np.einsum("hqd,hkd->hqk", qh, kh) * SCALE
        valid = mask[b] != 0
        if not valid.any():
            out[b] = bo
            continue
        att = np.where(valid[None, None, :], att, -np.inf)
        att = att - att.max(-1, keepdims=True)
        att = np.exp(att)
        att /= att.sum(-1, keepdims=True)
        o = np.einsum("hqk,hkd->hqd", att, vh)
        o = o.transpose(1, 0, 2).reshape(SQ, E)
        out[b] = o @ Wo.T + bo
    return out


def kernel(**inputs):
    global LAST_RESULTS
    q = np.asarray(inputs["q"], np.float32)
    k = np.asarray(inputs["k"], np.float32)
    v = np.asarray(inputs["v"], np.float32)
    mask = np.asarray(inputs["mask"])
    Wq, bq = np.asarray(inputs["Wq"], np.float32), np.asarray(inputs["bq"], np.float32)
    Wk, bk = np.asarray(inputs["Wk"], np.float32), np.asarray(inputs["bk"], np.float32)
    Wv, bv = np.asarray(inputs["Wv"], np.float32), np.asarray(inputs["bv"], np.float32)
    Wo, bo = np.asarray(inputs["Wo"], np.float32), np.asarray(inputs["bo"], np.float32)

    if any(np.abs(x).max() > 0 for x in (bq, bk, bv)):
        # q/k/v biases are zero in this problem's setup; a nonzero bias
        # would need the augmented-contraction path, so fall back.
        return _numpy_fallback(q, k, v, mask, Wq, bq, Wk, bk, Wv, bv, Wo, bo)

    valid = mask != 0
    counts = valid.sum(axis=1)
    if counts.max() == 0:
        return np.broadcast_to(bo, (B, SQ, E)).astype(np.float32).copy()

    skv = int(-(-counts.max() // 128) * 128)
    nc = build_program(skv)
    in_maps = make_in_maps(q, k, v, mask, Wq, Wk, Wv, Wo, skv)

    res = bass_utils.run_bass_kernel_spmd(nc, in_maps, core_ids=list(range(N_CORES)))
    LAST_RESULTS = res

    out = np.empty((B, SQ, E), np.float32)
    for b in range(B):
        if counts[b] == 0:
            out[b] = bo
        else:
            p0 = np.asarray(res.results[2 * b]["outT"], np.float32)
            p1 = np.asarray(res.results[2 * b + 1]["outT"], np.float32)
            out[b] = p0.T + p1.T + bo
    return out
